# revision 1
# baseline (speedup 1.0000x reference)
"""GAT self-attention Trainium2 kernel.

Full inputs -> shard graphs over 8 NeuronCores -> full output.

Math (per graph n, reference reformulated):
  g_i = sigmoid(relu(q @ W1_i) @ W2_i)            [2d]
  u_i^L = W_i @ (g_i[:d] * a_i[:d])               [k]   (left projector)
  u_i^R = W_i @ (g_i[d:] * a_i[d:])               [k]   (right projector)
  left_i = X @ u_i^L ; right_i = X @ u_i^R        [E]
  score[i,j] = lrelu(left_t[i] + right_t[j]), t = adj[i,j]; -BIG if adj==0
  E = exp(score); rs = rowsum(E); Xs = X / rs[:,None]
  out = (E^T @ Xs) @ W_2          (== softmax(score)^T @ (X @ W_2))
"""
import numpy as np
from contextlib import ExitStack

import concourse.bass as bass
import concourse.tile as tile
from concourse import mybir, bacc
from concourse.masks import make_identity

F32 = mybir.dt.float32
F32R = mybir.dt.float32r
U8 = mybir.dt.uint8
I32 = mybir.dt.int32
AF = mybir.ActivationFunctionType
OP = mybir.AluOpType

N_CORES = 8
N, E, K, D = 64, 512, 512, 512   # graphs, entities, in_dim, out_dim
NG = N // N_CORES                # graphs per core
NT = 3                           # edge types
P = 128
EC = E // P                      # 4 partition chunks of E
KC = K // P
DC2 = (2 * D) // P               # 8 chunks of the 2d gate dim
NEG_BIG = -200.0
LRELU_SLOPE = 0.2
USE_HW_LRELU = True   # ACT Lrelu not implemented in CoreSim; set False for sim runs



def _dma_split(nc, dst, src, pieces):
    """Split a big load along the leading src dim across sync/scalar queues."""
    n0 = dst.shape[1]
    step = max(1, n0 // pieces)
    engs = [nc.sync, nc.scalar]
    i = 0
    c = 0
    while i < n0:
        j = min(n0, i + step)
        engs[c % 2].dma_start(dst[:, i:j], src[:, i:j])
        i = j
        c += 1

def build(nc, reps=1):
    x = nc.dram_tensor("x", [NG, E, K], F32R, kind="ExternalInput").ap()
    adj = nc.dram_tensor("adj", [NG, E, E], I32, kind="ExternalInput").ap()
    qv = nc.dram_tensor("qv", [NG, K], F32R, kind="ExternalInput").ap()
    Wt = nc.dram_tensor("Wt", [NT, K, D], F32R, kind="ExternalInput").ap()
    at = nc.dram_tensor("at", [NT, 2 * D], F32, kind="ExternalInput").ap()
    W1 = nc.dram_tensor("W1", [NT, K, 2 * D], F32R, kind="ExternalInput").ap()
    W2q = nc.dram_tensor("W2q", [NT, 2 * D, 2 * D], F32R, kind="ExternalInput").ap()
    out = nc.dram_tensor("out", [NG, E, D], F32, kind="ExternalOutput").ap()
    nc._gat_io = (x, adj, qv, Wt, at, W1, W2q, out)

    _build_once(nc, reps)


def _build_once(nc, reps=1):
    x, adj, qv, Wt, at, W1, W2q, out = nc._gat_io
    with tile.TileContext(nc) as tc, ExitStack() as ctx:
        # ---------------- persistent pools ----------------
        pers = ctx.enter_context(tc.tile_pool(name="pers", bufs=1))
        ident = pers.tile([P, P], F32)
        make_identity(nc, ident[:])
        ones_stage = pers.tile([1, E], F32)
        nc.vector.memset(ones_stage[:], 1.0)
        ones_row = pers.tile([1, E], F32R)
        nc.vector.tensor_copy(ones_row[:], ones_stage[:])
        neg_col = pers.tile([P, 1], F32)
        nc.vector.memset(neg_col[:], NEG_BIG)
        # U_all[k%128, kc, c, n]: c in 0..2 -> left type c+1, 3..5 -> right
        U_all = pers.tile([P, KC, 2 * NT, NG], F32R)
        Wt2_sb = pers.tile([P, KC, D], F32R)
        _dma_split(nc, Wt2_sb[:], Wt[2].rearrange("(c p) d -> p c d", p=P), 2)

        # ---------------- prep phase ----------------
        def run_prep():
          with tc.tile_pool(name="prep", bufs=1) as prep:
            # qT[k%128, kc, n] via PE transposes of the natural [NG, K] layout
            qv_nat = prep.tile([NG, K], F32R)
            nc.sync.dma_start(qv_nat[:], qv)
            qT = prep.tile([P, KC, NG], F32R)
            for kc in range(KC):
                qps = ps_v.tile([P, NG], F32, tag="v")
                nc.tensor.transpose(
                    qps[:], qv_nat[:, kc * P:(kc + 1) * P].bitcast(F32), ident[:NG, :NG])
                nc.vector.tensor_copy(qT[:, kc, :], qps[:])
            # aT[d2%128, dc2]  (2d = 1024)
            aT = prep.tile([P, DC2, NT], F32)
            with nc.allow_non_contiguous_dma(reason="small aT load"):
                for t in range(NT):
                    nc.sync.dma_start(aT[:, :, t:t + 1],
                                      at[t].rearrange("(c p) -> p c", p=P)[:, :, None])

            for i in range(NT):
                # rrT = relu(W1_i^T @ qT): [2d, NG] laid out [128, DC2, NG]
                rrT = prep.tile([P, DC2, NG], F32R, tag="rrT")
                for whalf in range(2):
                    W1_sb = prep.tile([P, KC, D], F32R, tag="w1")
                    _dma_split(nc, W1_sb[:],
                               W1[i, :, whalf * D:(whalf + 1) * D].rearrange(
                                   "(c p) f -> p c f", p=P), 4)
                    for oc in range(DC2 // 2):
                        oc_g = whalf * (DC2 // 2) + oc
                        pps = ps_v.tile([P, NG], F32, tag="v")
                        for kc in range(KC):
                            nc.tensor.matmul(
                                pps[:], W1_sb[:, kc, oc * P:(oc + 1) * P],
                                qT[:, kc, :],
                                start=(kc == 0), stop=(kc == KC - 1))
                        nc.scalar.activation(rrT[:, oc_g, :], pps[:], AF.Relu)
                # gT = sigmoid(W2q_i^T @ rrT), W2q loaded in two out-halves
                gvT = prep.tile([P, DC2, NG], F32, tag="gvT")
                for half in range(2):
                    W2_sb = prep.tile([P, DC2, D], F32R, tag="w2")
                    _dma_split(
                        nc, W2_sb[:],
                        W2q[i, :, half * D:(half + 1) * D].rearrange(
                            "(c p) f -> p c f", p=P), 4)
                    for oc in range(DC2 // 2):
                        oc_g = half * (DC2 // 2) + oc
                        pps = ps_v.tile([P, NG], F32, tag="v")
                        for dc in range(DC2):
                            nc.tensor.matmul(
                                pps[:], W2_sb[:, dc, oc * P:(oc + 1) * P],
                                rrT[:, dc, :],
                                start=(dc == 0), stop=(dc == DC2 - 1))
                        nc.scalar.activation(gvT[:, oc_g, :], pps[:], AF.Sigmoid)
                # vT = gT * aT_i  (per-element over the 2d axis, bcast over n)
                vT = prep.tile([P, DC2, NG], F32R, tag="vT")
                nc.vector.tensor_tensor(
                    vT[:], gvT[:], aT[:, :, i:i + 1].broadcast_to((P, DC2, NG)),
                    OP.mult)
                # WT_i = W_i^T via PE transposes: [d%128, dc, k]
                W_sb = prep.tile([P, KC, D], F32R, tag="wsb")
                _dma_split(nc, W_sb[:], Wt[i].rearrange("(c p) d -> p c d", p=P), 2)
                WTi = prep.tile([P, EC, K], F32R, tag="wti")
                for dc in range(EC):
                    tps = ps_tr.tile([P, E], F32, tag="tr")
                    for kc in range(KC):
                        nc.tensor.transpose(
                            tps[:, kc * P:(kc + 1) * P],
                            W_sb[:, kc, dc * P:(dc + 1) * P].bitcast(F32), ident[:])
                    nc.vector.tensor_copy(WTi[:, dc, :], tps[:])
                # U_i(side) = W_i @ v-half : contraction over d
                for s in range(2):
                    ups = ps_v.tile([P, KC, NG], F32, tag="v")
                    for kc in range(KC):
                        for dc in range(EC):
                            nc.tensor.matmul(
                                ups[:, kc, :],
                                WTi[:, dc, kc * P:(kc + 1) * P],
                                vT[:, s * EC + dc, :],
                                start=(dc == 0), stop=(dc == EC - 1))
                    # c index: left types at 0..2, right at 3..5 (c = 3*s + i)
                    nc.vector.tensor_copy(U_all[:, :, 3 * s + i, :], ups[:])

        # ---------------- main per-graph pipeline ----------------
        sbuf = ctx.enter_context(tc.tile_pool(name="sbuf", bufs=2))
        deep = ctx.enter_context(tc.tile_pool(name="deep", bufs=3))
        small = ctx.enter_context(tc.tile_pool(name="small", bufs=2))
        one = ctx.enter_context(tc.tile_pool(name="one", bufs=1))
        ps_big = ctx.enter_context(tc.tile_pool(name="ps_big", bufs=2, space="PSUM"))
        ps_v = ctx.enter_context(tc.tile_pool(name="ps_v", bufs=4, space="PSUM"))
        ps_tr = ctx.enter_context(tc.tile_pool(name="ps_tr", bufs=1, space="PSUM"))
        ps_lr = ctx.enter_context(tc.tile_pool(name="ps_lr", bufs=1, space="PSUM"))

        def phase1(n):
            """front half: inputs, Xt, LR rows, stacks, masks"""
            X_sb = deep.tile([P, EC, K], F32R, tag="X")
            nc.sync.dma_start(X_sb[:, 0:2], x[n].rearrange("(c p) k -> p c k", p=P)[:, 0:2])
            nc.scalar.dma_start(X_sb[:, 2:4], x[n].rearrange("(c p) k -> p c k", p=P)[:, 2:4])
            adj_sb = sbuf.tile([P, EC, E], I32, tag="adj")
            nc.scalar.dma_start(adj_sb[:, 0:2], adj[n].rearrange("(c p) j -> p c j", p=P)[:, 0:2])
            nc.sync.dma_start(adj_sb[:, 2:4], adj[n].rearrange("(c p) j -> p c j", p=P)[:, 2:4])

            Xt_sb = sbuf.tile([P, KC, E], F32R, tag="Xt")
            for kc in range(KC):
                tps = ps_tr.tile([P, E], F32, tag="tr")
                for ec in range(EC):
                    nc.tensor.transpose(
                        tps[:, ec * P:(ec + 1) * P],
                        X_sb[:, ec, kc * P:(kc + 1) * P].bitcast(F32), ident[:])
                nc.scalar.copy(Xt_sb[:, kc, :], tps[:])

            pLR = ps_lr.tile([2 * NT, E], F32, tag="lr")
            for kc in range(KC):
                nc.tensor.matmul(pLR[:], U_all[:, kc, :, n], Xt_sb[:, kc, :],
                                 start=(kc == 0), stop=(kc == KC - 1))
            LR_sb = small.tile([2 * NT, E], F32R, tag="lrs")
            nc.scalar.copy(LR_sb[:], pLR[:])

            lhsT = []
            rhsT = []
            for t in range(NT):
                eng_a = nc.sync if t % 2 == 0 else nc.scalar
                eng_b = nc.scalar if t % 2 == 0 else nc.sync
                lt = small.tile([2, E], F32R, tag=f"lt{t}")
                eng_a.dma_start(lt[0:1, :], ones_row[:])
                eng_b.dma_start(lt[1:2, :], LR_sb[t:t + 1, :])
                rt = small.tile([2, E], F32R, tag=f"rt{t}")
                eng_a.dma_start(rt[0:1, :], LR_sb[NT + t:NT + t + 1, :])
                eng_b.dma_start(rt[1:2, :], ones_row[:])
                lhsT.append(lt)
                rhsT.append(rt)

            m0 = sbuf.tile([P, EC, E], U8, tag="m0")
            m2 = sbuf.tile([P, EC, E], U8, tag="m2")
            m3 = sbuf.tile([P, EC, E], U8, tag="m3")
            for h in range(2):
                sl = slice(2 * h, 2 * h + 2)
                nc.gpsimd.tensor_scalar(m2[:, sl], adj_sb[:, sl], 2, None, OP.is_equal)
                nc.gpsimd.tensor_scalar(m3[:, sl], adj_sb[:, sl], 3, None, OP.is_equal)
                nc.gpsimd.tensor_scalar(m0[:, sl], adj_sb[:, sl], 0, None, OP.is_equal)
            return dict(X_sb=X_sb, lhsT=lhsT, rhsT=rhsT, m0=m0, m2=m2, m3=m3)

        def phase2(n, st):
            """back half: select, exp, F, out"""
            X_sb = st["X_sb"]; lhsT = st["lhsT"]; rhsT = st["rhsT"]
            m0 = st["m0"]; m2 = st["m2"]; m3 = st["m3"]
            E_sb = deep.tile([P, EC, E], F32R, tag="E")
            rs = small.tile([P, EC], F32, tag="rs")
            for ic in range(EC):
                pv = []
                for t in range(NT):
                    pvt = ps_v.tile([P, E], F32, tag="v")
                    nc.tensor.matmul(pvt[:], lhsT[t][:, ic * P:(ic + 1) * P],
                                     rhsT[t][:], start=True, stop=True)
                    pv.append(pvt)
                nc.vector.copy_predicated(pv[0][:], m2[:, ic, :], pv[1][:])
                nc.vector.copy_predicated(pv[0][:], m3[:, ic, :], pv[2][:])
                nc.vector.copy_predicated(pv[0][:], m0[:, ic, :],
                                          neg_col[:, 0:1].broadcast_to((P, E)))
                ab = small.tile([P, E], F32, tag="ab")
                nc.scalar.activation(ab[:], pv[0][:], AF.Abs, scale=0.4)
                sc = small.tile([P, E], F32, tag="sc")
                nc.vector.scalar_tensor_tensor(sc[:], pv[0][:], 0.6, ab[:],
                                               OP.mult, OP.add)
                nc.scalar.activation(E_sb[:, ic, :], sc[:], AF.Exp,
                                     accum_out=rs[:, ic:ic + 1])
                rsr_ic = small.tile([P, EC], F32, tag="rsr")
                nc.vector.reciprocal(rsr_ic[:, ic:ic + 1], rs[:, ic:ic + 1])
                nc.vector.tensor_scalar(E_sb[:, ic, :], E_sb[:, ic, :].bitcast(F32),
                                        rsr_ic[:, ic:ic + 1], None, OP.mult)

            F_sb = sbuf.tile([P, KC, E], F32R, tag="F")
            for kc in range(KC):
                pF = ps_big.tile([P, E], F32, tag="big")
                for ec in range(EC):
                    nc.tensor.matmul(pF[:], X_sb[:, ec, kc * P:(kc + 1) * P],
                                     E_sb[:, ec, :],
                                     start=(ec == 0), stop=(ec == EC - 1))
                nc.scalar.copy(F_sb[:, kc, :], pF[:])

            for jc in range(EC):
                pO = ps_big.tile([P, D], F32, tag="big")
                for kc in range(KC):
                    nc.tensor.matmul(pO[:], F_sb[:, kc, jc * P:(jc + 1) * P],
                                     Wt2_sb[:, kc, :],
                                     start=(kc == 0), stop=(kc == KC - 1))
                o_sb = small.tile([P, D], F32, tag="osb")
                nc.scalar.copy(o_sb[:], pO[:])
                (nc.sync if jc % 2 == 0 else nc.scalar).dma_start(
                    out[n, jc * P:(jc + 1) * P, :], o_sb[:])

        def body_all(_iv=None):
          run_prep()
          for n in range(NG):
              phase2(n, phase1(n))

        if reps == 1:
            body_all()
        else:
            with tc.For_i(0, reps, 1) as _iv:
                body_all(_iv)
    return nc


_NC_CACHE = {}
TRACE = False
_LAST = {}


def _get_nc():
    if "nc" not in _NC_CACHE:
        nc = bacc.Bacc("TRN2", target_bir_lowering=False, debug=False)
        build(nc)
        nc.compile()
        _NC_CACHE["nc"] = nc
    return _NC_CACHE["nc"]


def kernel(input_state, adj, entity_mask, query_vec, W_type, a_type,
           qattn_W1, qattn_W2):
    from concourse import bass_utils
    nc = _get_nc()
    input_state = np.ascontiguousarray(input_state, dtype=np.float32)
    adj = np.ascontiguousarray(adj, dtype=np.int32)
    query_vec = np.ascontiguousarray(query_vec, dtype=np.float32)
    W_type = np.ascontiguousarray(W_type, dtype=np.float32)
    a_type = np.ascontiguousarray(a_type, dtype=np.float32)
    qattn_W1 = np.ascontiguousarray(qattn_W1, dtype=np.float32)
    qattn_W2 = np.ascontiguousarray(qattn_W2, dtype=np.float32)

    in_maps = []
    for c in range(N_CORES):
        sl = slice(c * NG, (c + 1) * NG)
        in_maps.append({
            "x": input_state[sl], "adj": adj[sl], "qv": query_vec[sl],
            "Wt": W_type, "at": a_type, "W1": qattn_W1, "W2q": qattn_W2,
        })
    res = bass_utils.run_bass_kernel_spmd(nc, in_maps, core_ids=list(range(N_CORES)),
                                          trace=TRACE, stitch_traces=TRACE)
    _LAST["exec_ns"] = res.exec_time_ns
    _LAST["mean_ns"] = res.mean_exec_time_ns
    _LAST["trace"] = res.instructions_and_trace
    _LAST["scope_times"] = res.per_core_scope_times
    out = np.concatenate([r["out"] for r in res.results], axis=0)
    return out.astype(np.float32)



# revision 5
# speedup vs baseline: 1.5126x; 1.5126x over previous
"""GAT self-attention Trainium2 kernel (v2).

Full inputs -> shard graphs over 8 NeuronCores -> full output.

Math (per graph n, reference reformulated):
  g_i = sigmoid(relu(q @ W1_i) @ W2_i)            [2d]
  u_i^L = W_i @ (g_i[:d] * a_i[:d])               [k]   (left projector)
  u_i^R = W_i @ (g_i[d:] * a_i[d:])               [k]   (right projector)
  left_i = X @ u_i^L ; right_i = X @ u_i^R        [E]
  score[i,j] = lrelu(left_t[i] + right_t[j]), t = adj[i,j]; -BIG if adj==0
  Ex = exp(score); rs = rowsum(Ex)
  out = Ex^T @ (X @ W_2 / rs[:,None])             (== softmax(score)^T @ (X @ W_2))

Host staging: x/q/W transposed + bf16; adj one-hot masks as uint8.
Device layout trick: the LR matmul emits a 12-row block
  [L1, 1, L2, 1, L3, 1, 1, R1, 1, R2, 1, R3]
(ones rows filled by a rank-1 "needle" accumulation), so each type's
outer-sum score matmul reads its [L_t; 1] / [1; R_t] operand pair
directly -- no staging copies or DMAs.
"""
import numpy as np
from contextlib import ExitStack

import ml_dtypes

import concourse.bass as bass
import concourse.tile as tile
from concourse import mybir, bacc

F32 = mybir.dt.float32
BF16 = mybir.dt.bfloat16
U8 = mybir.dt.uint8
AF = mybir.ActivationFunctionType
OP = mybir.AluOpType

N_CORES = 8
N, E, K, D = 64, 512, 512, 512   # graphs, entities, in_dim, out_dim
NG = N // N_CORES                # graphs per core
NT = 3                           # edge types
P = 128
EC = E // P                      # 4 partition chunks of E
KC = K // P
DC = D // P
TD2 = 2 * D
DC2 = TD2 // P                   # 8 chunks of the 2d gate dim
NEG_BIG = -200.0
LRELU_SLOPE = 0.2


def _dma_split(nc, dst, src, pieces):
    """Split a big load along dim 1 across sync/scalar queues."""
    n0 = dst.shape[1]
    step = max(1, n0 // pieces)
    engs = [nc.sync, nc.scalar]
    i = 0
    c = 0
    while i < n0:
        j = min(n0, i + step)
        engs[c % 2].dma_start(dst[:, i:j], src[:, i:j])
        i = j
        c += 1


def build(nc, reps=1):
    xT = nc.dram_tensor("xT", [NG, K, E], BF16, kind="ExternalInput").ap()
    masks = nc.dram_tensor("masks", [NG, 3, E, E], U8, kind="ExternalInput").ap()
    qT = nc.dram_tensor("qT", [K, NG], BF16, kind="ExternalInput").ap()
    at = nc.dram_tensor("at", [P, DC2, NT], F32, kind="ExternalInput").ap()
    WtT = nc.dram_tensor("WtT", [NT, D, K], BF16, kind="ExternalInput").ap()
    Wt2 = nc.dram_tensor("Wt2", [K, D], BF16, kind="ExternalInput").ap()
    W1 = nc.dram_tensor("W1", [NT, K, TD2], BF16, kind="ExternalInput").ap()
    W2q = nc.dram_tensor("W2q", [NT, TD2, TD2], BF16, kind="ExternalInput").ap()
    out = nc.dram_tensor("out", [NG, E, D], BF16, kind="ExternalOutput").ap()
    nc._gat_io = (xT, masks, qT, at, WtT, Wt2, W1, W2q, out)
    _build_once(nc, reps)


def _build_once(nc, reps=1):
    xT, masks, qT, at, WtT, Wt2, W1, W2q, out = nc._gat_io
    with tile.TileContext(nc) as tc, ExitStack() as ctx:
        # ---------------- persistent ----------------
        pers = ctx.enter_context(tc.tile_pool(name="pers", bufs=1))
        ones_row = pers.tile([1, E], BF16)
        nc.vector.memset(ones_row[:], 1.0)
        # needle: ones at rows {1,3,5,6,8,10} of the 12-row LR block
        needle = pers.tile([1, 12], BF16)
        nc.vector.memset(needle[:], 0.0)
        nc.vector.memset(needle[0:1, 1:6:2], 1.0)
        nc.vector.memset(needle[0:1, 6:11:2], 1.0)
        # U_all[k%128, kc, c, n]: c=2t -> u_t^L ; c=7+2t -> u_t^R ; rest 0
        U_all = pers.tile([P, KC, 12, NG], BF16)
        nc.gpsimd.memset(U_all[:], 0.0)
        Wt2_sb = pers.tile([P, KC, D], BF16)
        _dma_split(nc, Wt2_sb[:], Wt2.rearrange("(c p) d -> p c d", p=P), 2)
        qT_sb = pers.tile([P, KC, NG], BF16)
        nc.sync.dma_start(qT_sb[:], qT.rearrange("(c p) n -> p c n", p=P))
        at_sb = pers.tile([P, DC2, NT], F32)
        nc.sync.dma_start(at_sb[:], at)

        # ---------------- pools ----------------
        sbuf = ctx.enter_context(tc.tile_pool(name="sbuf", bufs=2))
        small = ctx.enter_context(tc.tile_pool(name="small", bufs=3))
        ps_v = ctx.enter_context(tc.tile_pool(name="ps_v", bufs=4, space="PSUM"))
        ps_big = ctx.enter_context(tc.tile_pool(name="ps_big", bufs=3, space="PSUM"))
        ps_lr = ctx.enter_context(tc.tile_pool(name="ps_lr", bufs=1, space="PSUM"))

        # ---------------- prep: gates -> U vectors ----------------
        def run_prep():
          with tc.tile_pool(name="prep", bufs=1) as prep:
            for i in range(NT):
                W1_sb = prep.tile([P, KC, TD2], BF16, tag="w1")
                _dma_split(nc, W1_sb[:],
                           W1[i].rearrange("(c p) f -> p c f", p=P), 2)
                # rrT[o2%128, oc2, n] = relu(W1_i^T @ q^T)
                rrT = prep.tile([P, DC2, NG], BF16, tag="rrT")
                for oc in range(DC2):
                    pps = ps_v.tile([P, NG], F32, tag="v")
                    for kc in range(KC):
                        nc.tensor.matmul(
                            pps[:], W1_sb[:, kc, oc * P:(oc + 1) * P],
                            qT_sb[:, kc, :],
                            start=(kc == 0), stop=(kc == KC - 1))
                    nc.scalar.activation(rrT[:, oc, :], pps[:], AF.Relu)
                W2_sb = prep.tile([P, DC2, TD2], BF16, tag="w2")
                _dma_split(nc, W2_sb[:],
                           W2q[i].rearrange("(c p) f -> p c f", p=P), 4)
                # gvT = sigmoid(W2q_i^T @ rrT)
                gvT = prep.tile([P, DC2, NG], F32, tag="gvT")
                for oc in range(DC2):
                    pps = ps_v.tile([P, NG], F32, tag="v")
                    for dc in range(DC2):
                        nc.tensor.matmul(
                            pps[:], W2_sb[:, dc, oc * P:(oc + 1) * P],
                            rrT[:, dc, :],
                            start=(dc == 0), stop=(dc == DC2 - 1))
                    nc.scalar.activation(gvT[:, oc, :], pps[:], AF.Sigmoid)
                # vT = gvT * a_i  (broadcast over n)
                vT = prep.tile([P, DC2, NG], BF16, tag="vT")
                nc.vector.tensor_tensor(
                    vT[:], gvT[:], at_sb[:, :, i:i + 1].broadcast_to((P, DC2, NG)),
                    OP.mult)
                # u_i^{L,R} = W_i @ v-half  (contraction over d, via W^T)
                WtT_sb = prep.tile([P, DC, K], BF16, tag="wtt")
                _dma_split(nc, WtT_sb[:],
                           WtT[i].rearrange("(c p) k -> p c k", p=P), 2)
                for s in range(2):
                    c = 2 * i if s == 0 else 7 + 2 * i
                    for kc in range(KC):
                        pu = ps_v.tile([P, NG], F32, tag="v")
                        for dc in range(DC):
                            nc.tensor.matmul(
                                pu[:], WtT_sb[:, dc, kc * P:(kc + 1) * P],
                                vT[:, s * DC + dc, :],
                                start=(dc == 0), stop=(dc == DC - 1))
                        nc.vector.tensor_copy(U_all[:, kc, c, :], pu[:])

        # ---------------- main per-graph pipeline ----------------
        def phase1(n):
            """inputs + the 12-row LR block"""
            Xt_sb = sbuf.tile([P, KC, E], BF16, tag="X")
            nc.sync.dma_start(Xt_sb[:], xT[n].rearrange("(c p) e -> p c e", p=P))
            m_sb = sbuf.tile([P, 3, EC, E], U8, tag="m")
            nc.sync.dma_start(m_sb[:], masks[n].rearrange("m (c p) j -> p m c j", p=P))

            pLR = ps_lr.tile([12, E], F32, tag="lr")
            for kc in range(KC):
                nc.tensor.matmul(pLR[:], U_all[:, kc, :, n], Xt_sb[:, kc, :],
                                 start=(kc == 0), stop=False)
            nc.tensor.matmul(pLR[:], needle[:], ones_row[:],
                             start=False, stop=True)
            LR_sb = small.tile([12, E], BF16, tag="lrs")
            nc.vector.tensor_copy(LR_sb[:], pLR[:])
            # Scatter the 12 rows to matmul-legal partition bases {0,32,64}:
            # AB[32t + q, 0:E]  = [L_t; 1]   (outer-sum lhsT for type t)
            # AB[32t + q, E:2E] = [1; R_t]   (outer-sum rhs  for type t)
            AB = small.tile([96, 2 * E], BF16, tag="ab")
            ABg = AB.rearrange("(g q) e2 -> g q e2", q=32)
            nc.sync.dma_start(
                ABg[:, 0:2, 0:E],
                LR_sb[0:6].rearrange("(g q) e -> g q e", q=2))
            nc.sync.dma_start(
                ABg[:, 0:2, E:2 * E],
                LR_sb[6:12].rearrange("(g q) e -> g q e", q=2))
            return dict(Xt_sb=Xt_sb, m_sb=m_sb, AB=AB)

        def phase2(n, st):
            """scores, softmax stats, H2, output"""
            Xt_sb = st["Xt_sb"]; m_sb = st["m_sb"]; AB = st["AB"]
            E_sb = sbuf.tile([P, EC, E], BF16, tag="E")
            rs = small.tile([P, EC], F32, tag="rs")
            rsr = small.tile([P, EC], F32, tag="rsr")
            for ic in range(EC):
                pv = []
                for t in range(NT):
                    pvt = ps_v.tile([P, E], F32, tag="v")
                    nc.tensor.matmul(
                        pvt[:], AB[32 * t:32 * t + 2, ic * P:(ic + 1) * P],
                        AB[32 * t:32 * t + 2, E:2 * E], start=True, stop=True)
                    pv.append(pvt)
                nc.vector.copy_predicated(pv[0][:], m_sb[:, 1, ic, :], pv[1][:])
                nc.vector.copy_predicated(pv[0][:], m_sb[:, 2, ic, :], pv[2][:])
                # S += -BIG * m0 ; then lrelu(x) = max(0.2x, x)
                nc.gpsimd.scalar_tensor_tensor(
                    pv[0][:], m_sb[:, 0, ic, :], NEG_BIG, pv[0][:], OP.mult, OP.add)
                nc.gpsimd.scalar_tensor_tensor(
                    pv[0][:], pv[0][:], LRELU_SLOPE, pv[0][:], OP.mult, OP.max)
                nc.scalar.activation(E_sb[:, ic, :], pv[0][:], AF.Exp,
                                     accum_out=rs[:, ic:ic + 1])
                nc.vector.reciprocal(rsr[:, ic:ic + 1], rs[:, ic:ic + 1])

            # H2 = X @ W_2, rows scaled by 1/rs during PSUM->SBUF copy
            H2s = sbuf.tile([P, EC, D], BF16, tag="H2")
            for ic in range(EC):
                pH = ps_big.tile([P, D], F32, tag="big")
                for kc in range(KC):
                    nc.tensor.matmul(pH[:], Xt_sb[:, kc, ic * P:(ic + 1) * P],
                                     Wt2_sb[:, kc, :],
                                     start=(kc == 0), stop=(kc == KC - 1))
                nc.gpsimd.tensor_scalar(H2s[:, ic, :], pH[:],
                                        rsr[:, ic:ic + 1], None, OP.mult)

            # out = Ex^T @ H2s
            for jc in range(EC):
                pO = ps_big.tile([P, D], F32, tag="big")
                for ic in range(EC):
                    nc.tensor.matmul(pO[:], E_sb[:, ic, jc * P:(jc + 1) * P],
                                     H2s[:, ic, :],
                                     start=(ic == 0), stop=(ic == EC - 1))
                o_sb = small.tile([P, D], BF16, tag="osb")
                nc.scalar.copy(o_sb[:], pO[:])
                nc.sync.dma_start(out[n, jc * P:(jc + 1) * P, :], o_sb[:])

        def body_all(_iv=None):
            run_prep()
            for n in range(NG):
                phase2(n, phase1(n))

        if reps == 1:
            body_all()
        else:
            with tc.For_i(0, reps, 1) as _iv:
                body_all(_iv)
    return nc


_NC_CACHE = {}
TRACE = False
_LAST = {}


def _get_nc():
    if "nc" not in _NC_CACHE:
        nc = bacc.Bacc("TRN2", target_bir_lowering=False, debug=False)
        build(nc)
        nc.compile()
        _NC_CACHE["nc"] = nc
    return _NC_CACHE["nc"]


def kernel(input_state, adj, entity_mask, query_vec, W_type, a_type,
           qattn_W1, qattn_W2):
    from concourse import bass_utils
    nc = _get_nc()
    bf = ml_dtypes.bfloat16
    input_state = np.asarray(input_state, dtype=np.float32)
    adj = np.asarray(adj, dtype=np.int32)
    query_vec = np.asarray(query_vec, dtype=np.float32)

    xT_all = np.ascontiguousarray(
        input_state.transpose(0, 2, 1)).astype(bf)              # [N, K, E]
    masks_all = np.ascontiguousarray(np.stack(
        [(adj == 0), (adj == 2), (adj == 3)], axis=1)).astype(np.uint8)
    qT_all = np.ascontiguousarray(query_vec.T).astype(bf)       # [K, N]
    at_h = np.ascontiguousarray(
        np.asarray(a_type, np.float32).reshape(NT, DC2, P).transpose(2, 1, 0))
    WtT_h = np.ascontiguousarray(
        np.asarray(W_type, np.float32).transpose(0, 2, 1)).astype(bf)
    Wt2_h = np.ascontiguousarray(np.asarray(W_type, np.float32)[2]).astype(bf)
    W1_h = np.ascontiguousarray(np.asarray(qattn_W1, np.float32)).astype(bf)
    W2q_h = np.ascontiguousarray(np.asarray(qattn_W2, np.float32)).astype(bf)

    in_maps = []
    for c in range(N_CORES):
        sl = slice(c * NG, (c + 1) * NG)
        in_maps.append({
            "xT": xT_all[sl], "masks": masks_all[sl],
            "qT": np.ascontiguousarray(qT_all[:, sl]),
            "at": at_h, "WtT": WtT_h, "Wt2": Wt2_h,
            "W1": W1_h, "W2q": W2q_h,
        })
    res = bass_utils.run_bass_kernel_spmd(nc, in_maps, core_ids=list(range(N_CORES)),
                                          trace=TRACE, stitch_traces=TRACE)
    _LAST["exec_ns"] = res.exec_time_ns
    _LAST["mean_ns"] = res.mean_exec_time_ns
    _LAST["trace"] = res.instructions_and_trace
    _LAST["scope_times"] = res.per_core_scope_times
    out = np.concatenate([np.asarray(r["out"]) for r in res.results], axis=0)
    return out.astype(np.float32)


# revision 11
# speedup vs baseline: 1.5604x; 1.0316x over previous
"""GAT self-attention Trainium2 kernel (v2).

Full inputs -> shard graphs over 8 NeuronCores -> full output.

Math (per graph n, reference reformulated):
  g_i = sigmoid(relu(q @ W1_i) @ W2_i)            [2d]
  u_i^L = W_i @ (g_i[:d] * a_i[:d])               [k]   (left projector)
  u_i^R = W_i @ (g_i[d:] * a_i[d:])               [k]   (right projector)
  left_i = X @ u_i^L ; right_i = X @ u_i^R        [E]
  score[i,j] = lrelu(left_t[i] + right_t[j]), t = adj[i,j]; -BIG if adj==0
  Ex = exp(score); rs = rowsum(Ex)
  out = Ex^T @ (X @ W_2 / rs[:,None])             (== softmax(score)^T @ (X @ W_2))

Host staging: x/q/W transposed + bf16; adj one-hot masks as uint8.
Device layout trick: the LR matmul emits a 12-row block
  [L1, 1, L2, 1, L3, 1, 1, R1, 1, R2, 1, R3]
(ones rows filled by a rank-1 "needle" accumulation), so each type's
outer-sum score matmul reads its [L_t; 1] / [1; R_t] operand pair
directly -- no staging copies or DMAs.
"""
import numpy as np
from contextlib import ExitStack

import ml_dtypes

import concourse.bass as bass
import concourse.tile as tile
from concourse import mybir, bacc
from concourse.masks import make_identity

F32 = mybir.dt.float32
BF16 = mybir.dt.bfloat16
U8 = mybir.dt.uint8
AF = mybir.ActivationFunctionType
OP = mybir.AluOpType

N_CORES = 8
N, E, K, D = 64, 512, 512, 512   # graphs, entities, in_dim, out_dim
NG = N // N_CORES                # graphs per core
NT = 3                           # edge types
P = 128
EC = E // P                      # 4 partition chunks of E
KC = K // P
DC = D // P
TD2 = 2 * D
DC2 = TD2 // P                   # 8 chunks of the 2d gate dim
NEG_BIG = -200.0
LRELU_SLOPE = 0.2


def _dma_split(nc, dst, src, pieces):
    """Split a big load along dim 1 across sync/scalar queues."""
    n0 = dst.shape[1]
    step = max(1, n0 // pieces)
    engs = [nc.sync, nc.scalar]
    i = 0
    c = 0
    while i < n0:
        j = min(n0, i + step)
        engs[c % 2].dma_start(dst[:, i:j], src[:, i:j])
        i = j
        c += 1


def build(nc, reps=1):
    xT = nc.dram_tensor("xT", [NG, K, E], BF16, kind="ExternalInput").ap()
    masks = nc.dram_tensor("masks", [NG, 3, E, E], U8, kind="ExternalInput").ap()
    qT = nc.dram_tensor("qT", [K, NG], BF16, kind="ExternalInput").ap()
    at = nc.dram_tensor("at", [P, DC2, NT], F32, kind="ExternalInput").ap()
    WtT = nc.dram_tensor("WtT", [NT, D, K], BF16, kind="ExternalInput").ap()
    Wt2 = nc.dram_tensor("Wt2", [K, D], BF16, kind="ExternalInput").ap()
    W1 = nc.dram_tensor("W1", [NT, K, TD2], BF16, kind="ExternalInput").ap()
    W2q = nc.dram_tensor("W2q", [NT, TD2, TD2], BF16, kind="ExternalInput").ap()
    out = nc.dram_tensor("out", [NG, E, D], BF16, kind="ExternalOutput").ap()
    nc._gat_io = (xT, masks, qT, at, WtT, Wt2, W1, W2q, out)
    _build_once(nc, reps)


def _build_once(nc, reps=1):
    xT, masks, qT, at, WtT, Wt2, W1, W2q, out = nc._gat_io
    with tile.TileContext(nc) as tc, ExitStack() as ctx:
        # ---------------- persistent ----------------
        pers = ctx.enter_context(tc.tile_pool(name="pers", bufs=1))
        ident = pers.tile([P, P], F32)
        make_identity(nc, ident[:])
        ident_bf = pers.tile([P, P], BF16)
        nc.vector.tensor_copy(ident_bf[:], ident[:])
        ones_row = pers.tile([1, E], BF16)
        nc.vector.memset(ones_row[:], 1.0)
        # needle: ones at rows {1,3,5,6,8,10} of the 12-row LR block
        needle = pers.tile([1, 12], BF16)
        nc.vector.memset(needle[:], 0.0)
        nc.vector.memset(needle[0:1, 1:6:2], 1.0)
        nc.vector.memset(needle[0:1, 6:11:2], 1.0)
        # U_all[k%128, kc, c, n]: c=2t -> u_t^L ; c=7+2t -> u_t^R ; rest 0
        U_all = pers.tile([P, KC, 12, NG], BF16)
        nc.gpsimd.memset(U_all[:], 0.0)
        Wt2_sb = pers.tile([P, KC, D], BF16)
        _dma_split(nc, Wt2_sb[:], Wt2.rearrange("(c p) d -> p c d", p=P), 2)
        qT_sb = pers.tile([P, KC, NG], BF16)
        nc.sync.dma_start(qT_sb[:], qT.rearrange("(c p) n -> p c n", p=P))
        at_sb = pers.tile([P, DC2, NT], F32)
        nc.sync.dma_start(at_sb[:], at)

        # ---------------- pools ----------------
        sbuf = ctx.enter_context(tc.tile_pool(name="sbuf", bufs=2))
        small = ctx.enter_context(tc.tile_pool(name="small", bufs=3))
        ps_v = ctx.enter_context(tc.tile_pool(name="ps_v", bufs=4, space="PSUM"))
        ps_big = ctx.enter_context(tc.tile_pool(name="ps_big", bufs=3, space="PSUM"))
        ps_lr = ctx.enter_context(tc.tile_pool(name="ps_lr", bufs=1, space="PSUM"))

        # ---------------- prep: gates -> U vectors ----------------
        # All gate matmuls use the NG(=8)-row operand as the 128-col-max
        # stationary side and stream the big weight as the moving side, so
        # each stage is a handful of ap=512 matmuls instead of dozens of
        # ap=8 ones; small [8, .] results are transposed back on the PE.
        def run_prep():
          with tc.tile_pool(name="prep", bufs=1) as prep:
            for i in range(NT):
                W1_sb = prep.tile([P, KC, TD2], BF16, tag="w1")
                _dma_split(nc, W1_sb[:],
                           W1[i].rearrange("(c p) f -> p c f", p=P), 2)
                # rr[n, o2] = relu(q @ W1_i), two 512-col halves
                rr_sb = prep.tile([NG, TD2], BF16, tag="rr")
                for h in range(2):
                    pr = ps_big.tile([NG, D], F32, tag="big")
                    for kc in range(KC):
                        nc.tensor.matmul(
                            pr[:], qT_sb[:, kc, :],
                            W1_sb[:, kc, h * D:(h + 1) * D],
                            start=(kc == 0), stop=(kc == KC - 1))
                    nc.scalar.activation(rr_sb[:, h * D:(h + 1) * D], pr[:],
                                         AF.Relu)
                # rrT[o2%128, oc2, n] via PE transposes
                prT = ps_v.tile([P, DC2, NG], BF16, tag="v")
                for b in range(DC2):
                    nc.tensor.transpose(prT[:, b, :],
                                        rr_sb[:, b * P:(b + 1) * P],
                                        ident_bf[:NG, :NG])
                rrT = prep.tile([P, DC2, NG], BF16, tag="rrT")
                nc.vector.tensor_copy(rrT[:], prT[:])
                W2_sb = prep.tile([P, DC2, TD2], BF16, tag="w2")
                _dma_split(nc, W2_sb[:],
                           W2q[i].rearrange("(c p) f -> p c f", p=P), 4)
                # g[n, o2] = sigmoid(rr @ W2q_i)
                g_sb = prep.tile([NG, TD2], BF16, tag="g")
                for h in range(2):
                    pg = ps_big.tile([NG, D], F32, tag="big")
                    for dc in range(DC2):
                        nc.tensor.matmul(
                            pg[:], rrT[:, dc, :],
                            W2_sb[:, dc, h * D:(h + 1) * D],
                            start=(dc == 0), stop=(dc == DC2 - 1))
                    nc.scalar.activation(g_sb[:, h * D:(h + 1) * D], pg[:],
                                         AF.Sigmoid)
                # vT[o2%128, oc2, n] = g^T * a_i (a-mult fused into the copy)
                pgT = ps_v.tile([P, DC2, NG], BF16, tag="v")
                for b in range(DC2):
                    nc.tensor.transpose(pgT[:, b, :],
                                        g_sb[:, b * P:(b + 1) * P],
                                        ident_bf[:NG, :NG])
                vT = prep.tile([P, DC2, NG], BF16, tag="vT")
                nc.vector.tensor_tensor(
                    vT[:], pgT[:], at_sb[:, :, i:i + 1].broadcast_to((P, DC2, NG)),
                    OP.mult)
                # u_i^{L,R}[n, k] = v-half @ W_i^T, then transpose into U_all
                WtT_sb = prep.tile([P, DC, K], BF16, tag="wtt")
                _dma_split(nc, WtT_sb[:],
                           WtT[i].rearrange("(c p) k -> p c k", p=P), 2)
                for s in range(2):
                    c = 2 * i if s == 0 else 7 + 2 * i
                    pu = ps_big.tile([NG, K], F32, tag="big")
                    for dc in range(DC):
                        nc.tensor.matmul(
                            pu[:], vT[:, s * DC + dc, :], WtT_sb[:, dc, :],
                            start=(dc == 0), stop=(dc == DC - 1))
                    u_sb = prep.tile([NG, K], BF16, tag=f"u{s}")
                    nc.gpsimd.tensor_copy(u_sb[:], pu[:])
                    puT = ps_v.tile([P, KC, NG], BF16, tag="v")
                    for kc in range(KC):
                        nc.tensor.transpose(puT[:, kc, :],
                                            u_sb[:, kc * P:(kc + 1) * P],
                                            ident_bf[:NG, :NG])
                    nc.vector.tensor_copy(U_all[:, :, c, :], puT[:])

        # ---------------- main per-graph pipeline ----------------
        def phase1(n):
            """inputs + the 12-row LR block"""
            Xt_sb = sbuf.tile([P, KC, E], BF16, tag="X")
            nc.sync.dma_start(Xt_sb[:], xT[n].rearrange("(c p) e -> p c e", p=P))
            m_sb = sbuf.tile([P, 3, EC, E], U8, tag="m")
            nc.sync.dma_start(m_sb[:], masks[n].rearrange("m (c p) j -> p m c j", p=P))

            pLR = ps_lr.tile([12, E], F32, tag="lr")
            for kc in range(KC):
                nc.tensor.matmul(pLR[:], U_all[:, kc, :, n], Xt_sb[:, kc, :],
                                 start=(kc == 0), stop=False)
            nc.tensor.matmul(pLR[:], needle[:], ones_row[:],
                             start=False, stop=True)
            LR_sb = small.tile([12, E], BF16, tag="lrs")
            nc.vector.tensor_copy(LR_sb[:], pLR[:])
            # Scatter the 12 rows to matmul-legal partition bases {0,32,64}:
            # AB[32t + q, 0:E]  = [L_t; 1]   (outer-sum lhsT for type t)
            # AB[32t + q, E:2E] = [1; R_t]   (outer-sum rhs  for type t)
            AB = small.tile([96, 2 * E], BF16, tag="ab")
            ABg = AB.rearrange("(g q) e2 -> g q e2", q=32)
            nc.sync.dma_start(
                ABg[:, 0:2, 0:E],
                LR_sb[0:6].rearrange("(g q) e -> g q e", q=2))
            nc.sync.dma_start(
                ABg[:, 0:2, E:2 * E],
                LR_sb[6:12].rearrange("(g q) e -> g q e", q=2))
            return dict(Xt_sb=Xt_sb, m_sb=m_sb, AB=AB)

        def phase2(n, st):
            """scores, softmax stats, H2, output"""
            Xt_sb = st["Xt_sb"]; m_sb = st["m_sb"]; AB = st["AB"]
            E_sb = sbuf.tile([P, EC, E], BF16, tag="E")
            rs = small.tile([P, EC], F32, tag="rs")
            rsr = small.tile([P, EC], F32, tag="rsr")
            for ic in range(EC):
                pv = []
                for t in range(NT):
                    pvt = ps_v.tile([P, E], F32, tag="v")
                    nc.tensor.matmul(
                        pvt[:], AB[32 * t:32 * t + 2, ic * P:(ic + 1) * P],
                        AB[32 * t:32 * t + 2, E:2 * E], start=True, stop=True)
                    pv.append(pvt)
                nc.vector.copy_predicated(pv[0][:], m_sb[:, 1, ic, :], pv[1][:])
                nc.vector.copy_predicated(pv[0][:], m_sb[:, 2, ic, :], pv[2][:])
                # S += -BIG * m0 ; then lrelu(x) = max(0.2x, x)
                nc.gpsimd.scalar_tensor_tensor(
                    pv[0][:], m_sb[:, 0, ic, :], NEG_BIG, pv[0][:], OP.mult, OP.add)
                nc.gpsimd.scalar_tensor_tensor(
                    pv[0][:], pv[0][:], LRELU_SLOPE, pv[0][:], OP.mult, OP.max)
                nc.scalar.activation(E_sb[:, ic, :], pv[0][:], AF.Exp,
                                     accum_out=rs[:, ic:ic + 1])
                nc.vector.reciprocal(rsr[:, ic:ic + 1], rs[:, ic:ic + 1])

            # H2 = X @ W_2, rows scaled by 1/rs during PSUM->SBUF copy
            H2s = sbuf.tile([P, EC, D], BF16, tag="H2")
            for ic in range(EC):
                pH = ps_big.tile([P, D], F32, tag="big")
                for kc in range(KC):
                    nc.tensor.matmul(pH[:], Xt_sb[:, kc, ic * P:(ic + 1) * P],
                                     Wt2_sb[:, kc, :],
                                     start=(kc == 0), stop=(kc == KC - 1))
                nc.scalar.activation(H2s[:, ic, :], pH[:], AF.Copy,
                                     scale=rsr[:, ic:ic + 1])

            # out = Ex^T @ H2s
            for jc in range(EC):
                pO = ps_big.tile([P, D], F32, tag="big")
                for ic in range(EC):
                    nc.tensor.matmul(pO[:], E_sb[:, ic, jc * P:(jc + 1) * P],
                                     H2s[:, ic, :],
                                     start=(ic == 0), stop=(ic == EC - 1))
                o_sb = small.tile([P, D], BF16, tag="osb")
                if jc % 2 == 0:
                    nc.scalar.copy(o_sb[:], pO[:])
                else:
                    nc.vector.tensor_copy(o_sb[:], pO[:])
                nc.sync.dma_start(out[n, jc * P:(jc + 1) * P, :], o_sb[:])

        def body_all(_iv=None):
            run_prep()
            for n in range(NG):
                phase2(n, phase1(n))

        if reps == 1:
            body_all()
        else:
            with tc.For_i(0, reps, 1) as _iv:
                body_all(_iv)
    return nc


_NC_CACHE = {}
TRACE = False
_LAST = {}


def _get_nc():
    if "nc" not in _NC_CACHE:
        nc = bacc.Bacc("TRN2", target_bir_lowering=False, debug=False)
        build(nc)
        nc.compile()
        _NC_CACHE["nc"] = nc
    return _NC_CACHE["nc"]


def kernel(input_state, adj, entity_mask, query_vec, W_type, a_type,
           qattn_W1, qattn_W2):
    from concourse import bass_utils
    nc = _get_nc()
    bf = ml_dtypes.bfloat16
    input_state = np.asarray(input_state, dtype=np.float32)
    adj = np.asarray(adj, dtype=np.int32)
    query_vec = np.asarray(query_vec, dtype=np.float32)

    xT_all = np.ascontiguousarray(
        input_state.transpose(0, 2, 1)).astype(bf)              # [N, K, E]
    masks_all = np.ascontiguousarray(np.stack(
        [(adj == 0), (adj == 2), (adj == 3)], axis=1)).astype(np.uint8)
    qT_all = np.ascontiguousarray(query_vec.T).astype(bf)       # [K, N]
    at_h = np.ascontiguousarray(
        np.asarray(a_type, np.float32).reshape(NT, DC2, P).transpose(2, 1, 0))
    WtT_h = np.ascontiguousarray(
        np.asarray(W_type, np.float32).transpose(0, 2, 1)).astype(bf)
    Wt2_h = np.ascontiguousarray(np.asarray(W_type, np.float32)[2]).astype(bf)
    W1_h = np.ascontiguousarray(np.asarray(qattn_W1, np.float32)).astype(bf)
    W2q_h = np.ascontiguousarray(np.asarray(qattn_W2, np.float32)).astype(bf)

    in_maps = []
    for c in range(N_CORES):
        sl = slice(c * NG, (c + 1) * NG)
        in_maps.append({
            "xT": xT_all[sl], "masks": masks_all[sl],
            "qT": np.ascontiguousarray(qT_all[:, sl]),
            "at": at_h, "WtT": WtT_h, "Wt2": Wt2_h,
            "W1": W1_h, "W2q": W2q_h,
        })
    res = bass_utils.run_bass_kernel_spmd(nc, in_maps, core_ids=list(range(N_CORES)),
                                          trace=TRACE, stitch_traces=TRACE)
    _LAST["exec_ns"] = res.exec_time_ns
    _LAST["mean_ns"] = res.mean_exec_time_ns
    _LAST["trace"] = res.instructions_and_trace
    _LAST["scope_times"] = res.per_core_scope_times
    out = np.concatenate([np.asarray(r["out"]) for r in res.results], axis=0)
    return out.astype(np.float32)


# revision 19
# speedup vs baseline: 1.6919x; 1.0842x over previous
"""GAT self-attention Trainium2 kernel (v2).

Full inputs -> shard graphs over 8 NeuronCores -> full output.

Math (per graph n, reference reformulated):
  g_i = sigmoid(relu(q @ W1_i) @ W2_i)            [2d]
  u_i^L = W_i @ (g_i[:d] * a_i[:d])               [k]   (left projector)
  u_i^R = W_i @ (g_i[d:] * a_i[d:])               [k]   (right projector)
  left_i = X @ u_i^L ; right_i = X @ u_i^R        [E]
  score[i,j] = lrelu(left_t[i] + right_t[j]), t = adj[i,j]; -BIG if adj==0
  Ex = exp(score); rs = rowsum(Ex)
  out = Ex^T @ (X @ W_2 / rs[:,None])             (== softmax(score)^T @ (X @ W_2))

Host staging: x/q/W transposed + bf16; adj one-hot masks as uint8.
Device layout trick: the LR matmul emits a 12-row block
  [L1, 1, L2, 1, L3, 1, 1, R1, 1, R2, 1, R3]
(ones rows filled by a rank-1 "needle" accumulation), so each type's
outer-sum score matmul reads its [L_t; 1] / [1; R_t] operand pair
directly -- no staging copies or DMAs.
"""
import numpy as np
from contextlib import ExitStack

import ml_dtypes

import concourse.bass as bass
import concourse.tile as tile
from concourse import mybir, bacc
from concourse.masks import make_identity

F32 = mybir.dt.float32
BF16 = mybir.dt.bfloat16
U8 = mybir.dt.uint8
AF = mybir.ActivationFunctionType
OP = mybir.AluOpType

N_CORES = 8
N, E, K, D = 64, 512, 512, 512   # graphs, entities, in_dim, out_dim
NG = N // N_CORES                # graphs per core
NT = 3                           # edge types
P = 128
EC = E // P                      # 4 partition chunks of E
KC = K // P
DC = D // P
TD2 = 2 * D
DC2 = TD2 // P                   # 8 chunks of the 2d gate dim
NEG_BIG = -200.0
LRELU_SLOPE = 0.2


def _dma_split(nc, dst, src, pieces):
    """Split a big load along dim 1 across sync/scalar queues."""
    n0 = dst.shape[1]
    step = max(1, n0 // pieces)
    engs = [nc.sync, nc.scalar]
    i = 0
    c = 0
    while i < n0:
        j = min(n0, i + step)
        engs[c % 2].dma_start(dst[:, i:j], src[:, i:j])
        i = j
        c += 1


def build(nc, reps=1):
    xT = nc.dram_tensor("xT", [NG, K, E], BF16, kind="ExternalInput").ap()
    masks = nc.dram_tensor("masks", [NG, 3, E, E], U8, kind="ExternalInput").ap()
    qT = nc.dram_tensor("qT", [K, NG], BF16, kind="ExternalInput").ap()
    at = nc.dram_tensor("at", [P, DC2, NT], F32, kind="ExternalInput").ap()
    WtT = nc.dram_tensor("WtT", [NT, D, K], BF16, kind="ExternalInput").ap()
    Wt2 = nc.dram_tensor("Wt2", [K, D], BF16, kind="ExternalInput").ap()
    W1 = nc.dram_tensor("W1", [NT, K, TD2], BF16, kind="ExternalInput").ap()
    W2q = nc.dram_tensor("W2q", [NT, TD2, TD2], BF16, kind="ExternalInput").ap()
    out = nc.dram_tensor("out", [NG, E, D], BF16, kind="ExternalOutput").ap()
    nc._gat_io = (xT, masks, qT, at, WtT, Wt2, W1, W2q, out)
    _build_once(nc, reps)


def _build_once(nc, reps=1):
    xT, masks, qT, at, WtT, Wt2, W1, W2q, out = nc._gat_io
    with tile.TileContext(nc) as tc, ExitStack() as ctx:
        # ---------------- persistent ----------------
        pers = ctx.enter_context(tc.tile_pool(name="pers", bufs=1))
        ident = pers.tile([P, P], F32)
        make_identity(nc, ident[:])
        ident_bf = pers.tile([P, P], BF16)
        nc.vector.tensor_copy(ident_bf[:], ident[:])
        # U_all[k%128, kc, c, n]: c=t -> u_t^L ; c=3+t -> u_t^R
        U_all = pers.tile([P, KC, 6, NG], BF16)
        qT_sb = pers.tile([P, KC, NG], BF16)
        nc.sync.dma_start(qT_sb[:], qT.rearrange("(c p) n -> p c n", p=P))
        at_sb = pers.tile([P, DC2, NT], F32)
        nc.sync.dma_start(at_sb[:], at)
        # Persistent AB ring: ones rows at fixed spots, L/R rows DMA'd per
        # graph. AB[32t + q, 0:E] = [L_t; 1], AB[32t + q, E:2E] = [1; R_t].
        AB_ring = []
        for r in range(3):
            ab = pers.tile([96, 2 * E], BF16, tag=f"ab{r}")
            for t in range(NT):
                nc.vector.memset(ab[32 * t + 1:32 * t + 2, 0:E], 1.0)
                nc.gpsimd.memset(ab[32 * t:32 * t + 1, E:2 * E], 1.0)
            AB_ring.append(ab)
        Wt2_sb = pers.tile([P, KC, D], BF16)

        # ---------------- pools ----------------
        sbuf = ctx.enter_context(tc.tile_pool(name="sbuf", bufs=3))
        small = ctx.enter_context(tc.tile_pool(name="small", bufs=3))
        ps_v = ctx.enter_context(tc.tile_pool(name="ps_v", bufs=4, space="PSUM"))
        ps_big = ctx.enter_context(tc.tile_pool(name="ps_big", bufs=3, space="PSUM"))
        ps_lr = ctx.enter_context(tc.tile_pool(name="ps_lr", bufs=1, space="PSUM"))

        # ---------------- prep: gates -> U vectors ----------------
        # All gate matmuls use the NG(=8)-row operand as the 128-col-max
        # stationary side and stream the big weight as the moving side, so
        # each stage is a handful of ap=512 matmuls instead of dozens of
        # ap=8 ones; small [8, .] results are transposed back on the PE.
        def run_prep():
          with tc.tile_pool(name="prep", bufs=2) as prep:
            for i in range(NT):
                W1_sb = prep.tile([P, KC, TD2], BF16, tag="w1")
                _dma_split(nc, W1_sb[:],
                           W1[i].rearrange("(c p) f -> p c f", p=P), 2)
                # rr[n, o2] = relu(q @ W1_i), two 512-col halves
                rr_sb = prep.tile([NG, TD2], BF16, tag="rr")
                for h in range(2):
                    pr = ps_big.tile([NG, D], F32, tag="big")
                    for kc in range(KC):
                        nc.tensor.matmul(
                            pr[:], qT_sb[:, kc, :],
                            W1_sb[:, kc, h * D:(h + 1) * D],
                            start=(kc == 0), stop=(kc == KC - 1))
                    nc.scalar.activation(rr_sb[:, h * D:(h + 1) * D], pr[:],
                                         AF.Relu)
                # rrT[o2%128, oc2, n] via PE transposes
                prT = ps_v.tile([P, DC2, NG], BF16, tag="v")
                for b in range(DC2):
                    nc.tensor.transpose(prT[:, b, :],
                                        rr_sb[:, b * P:(b + 1) * P],
                                        ident_bf[:NG, :NG])
                rrT = prep.tile([P, DC2, NG], BF16, tag="rrT")
                nc.vector.tensor_copy(rrT[:], prT[:])
                W2_sb = prep.tile([P, DC2, TD2], BF16, tag="w2")
                _dma_split(nc, W2_sb[:],
                           W2q[i].rearrange("(c p) f -> p c f", p=P), 4)
                # g[n, o2] = sigmoid(rr @ W2q_i)
                g_sb = prep.tile([NG, TD2], BF16, tag="g")
                for h in range(2):
                    pg = ps_big.tile([NG, D], F32, tag="big")
                    for dc in range(DC2):
                        nc.tensor.matmul(
                            pg[:], rrT[:, dc, :],
                            W2_sb[:, dc, h * D:(h + 1) * D],
                            start=(dc == 0), stop=(dc == DC2 - 1))
                    nc.scalar.activation(g_sb[:, h * D:(h + 1) * D], pg[:],
                                         AF.Sigmoid)
                # vT[o2%128, dc, s, n] = g^T * a_i (a-mult fused into the
                # copy), (s, n) adjacent so both u-sides share one stationary
                pgT = ps_v.tile([P, DC2, NG], BF16, tag="v")
                for b in range(DC2):
                    nc.tensor.transpose(pgT[:, b, :],
                                        g_sb[:, b * P:(b + 1) * P],
                                        ident_bf[:NG, :NG])
                vT = prep.tile([P, DC, 2, NG], BF16, tag="vT")
                for s in range(2):
                    nc.vector.tensor_tensor(
                        vT[:, :, s, :], pgT[:, s * DC:(s + 1) * DC, :],
                        at_sb[:, s * DC:(s + 1) * DC, i:i + 1].broadcast_to(
                            (P, DC, NG)),
                        OP.mult)
                # u_i^{L,R}[n, k] = v-half @ W_i^T, both sides in one
                # 16-col stationary; transpose into U_all
                WtT_sb = prep.tile([P, DC, K], BF16, tag="wtt")
                _dma_split(nc, WtT_sb[:],
                           WtT[i].rearrange("(c p) k -> p c k", p=P), 2)
                pu = ps_big.tile([2 * NG, K], F32, tag="big")
                for dc in range(DC):
                    nc.tensor.matmul(
                        pu[:], vT[:, dc, :, :], WtT_sb[:, dc, :],
                        start=(dc == 0), stop=(dc == DC - 1))
                u_sb = prep.tile([2 * NG, K], BF16, tag="u")
                nc.gpsimd.tensor_copy(u_sb[:], pu[:])
                puT = ps_v.tile([P, KC, 2 * NG], BF16, tag="v")
                for kc in range(KC):
                    nc.tensor.transpose(puT[:, kc, :],
                                        u_sb[:, kc * P:(kc + 1) * P],
                                        ident_bf[:2 * NG, :2 * NG])
                nc.vector.tensor_copy(U_all[:, :, i, :], puT[:, :, 0:NG])
                nc.vector.tensor_copy(U_all[:, :, 3 + i, :], puT[:, :, NG:2 * NG])

        # ---------------- main per-graph pipeline ----------------
        def phase1(n):
            """inputs + the 12-row LR block"""
            Xt_sb = sbuf.tile([P, KC, E], BF16, tag="X")
            nc.sync.dma_start(Xt_sb[:], xT[n].rearrange("(c p) e -> p c e", p=P))
            m_sb = sbuf.tile([P, 3, EC, E], U8, tag="m")
            nc.sync.dma_start(m_sb[:], masks[n].rearrange("m (c p) j -> p m c j", p=P))

            pLR = ps_lr.tile([6, E], F32, tag="lr")
            for kc in range(KC):
                nc.tensor.matmul(pLR[:], U_all[:, kc, :, n], Xt_sb[:, kc, :],
                                 start=(kc == 0), stop=(kc == KC - 1))
            LR_sb = small.tile([6, E], BF16, tag="lrs")
            nc.vector.tensor_copy(LR_sb[:], pLR[:])
            # Scatter L/R rows to matmul-legal partition bases {0,32,64};
            # the ones rows are pre-set in the persistent ring tiles.
            AB = AB_ring[n % 3]
            ABg = AB.rearrange("(g q) e2 -> g q e2", q=32)
            nc.sync.dma_start(
                ABg[:, 0:1, 0:E],
                LR_sb[0:3].rearrange("(g q) e -> g q e", q=1))
            nc.sync.dma_start(
                ABg[:, 1:2, E:2 * E],
                LR_sb[3:6].rearrange("(g q) e -> g q e", q=1))
            return dict(Xt_sb=Xt_sb, m_sb=m_sb, AB=AB)

        def phase2(n, st):
            """scores, softmax stats, H2, output"""
            Xt_sb = st["Xt_sb"]; m_sb = st["m_sb"]; AB = st["AB"]
            E_sb = sbuf.tile([P, EC, E], BF16, tag="E")
            rs = small.tile([P, EC], F32, tag="rs")
            rsr = small.tile([P, EC], F32, tag="rsr")
            for ic in range(EC):
                pv = []
                for t in range(NT):
                    pvt = ps_v.tile([P, E], F32, tag="v")
                    nc.tensor.matmul(
                        pvt[:], AB[32 * t:32 * t + 2, ic * P:(ic + 1) * P],
                        AB[32 * t:32 * t + 2, E:2 * E], start=True, stop=True)
                    pv.append(pvt)
                nc.vector.copy_predicated(pv[0][:], m_sb[:, 1, ic, :], pv[1][:])
                nc.vector.copy_predicated(pv[0][:], m_sb[:, 2, ic, :], pv[2][:])
                # S += -BIG * m0 ; then lrelu(x) = max(0.2x, x)
                nc.gpsimd.scalar_tensor_tensor(
                    pv[0][:], m_sb[:, 0, ic, :], NEG_BIG, pv[0][:], OP.mult, OP.add)
                nc.gpsimd.scalar_tensor_tensor(
                    pv[0][:], pv[0][:], LRELU_SLOPE, pv[0][:], OP.mult, OP.max)
                nc.scalar.activation(E_sb[:, ic, :], pv[0][:], AF.Exp,
                                     accum_out=rs[:, ic:ic + 1])
                nc.vector.reciprocal(rsr[:, ic:ic + 1], rs[:, ic:ic + 1])

            # H2 = X @ W_2, rows scaled by 1/rs during PSUM->SBUF copy
            H2s = sbuf.tile([P, EC, D], BF16, tag="H2")
            for ic in range(EC):
                pH = ps_big.tile([P, D], F32, tag="big")
                for kc in range(KC):
                    nc.tensor.matmul(pH[:], Xt_sb[:, kc, ic * P:(ic + 1) * P],
                                     Wt2_sb[:, kc, :],
                                     start=(kc == 0), stop=(kc == KC - 1))
                nc.scalar.activation(H2s[:, ic, :], pH[:], AF.Copy,
                                     scale=rsr[:, ic:ic + 1])

            # out = Ex^T @ H2s
            for jc in range(EC):
                pO = ps_big.tile([P, D], F32, tag="big")
                for ic in range(EC):
                    nc.tensor.matmul(pO[:], E_sb[:, ic, jc * P:(jc + 1) * P],
                                     H2s[:, ic, :],
                                     start=(ic == 0), stop=(ic == EC - 1))
                o_sb = small.tile([P, D], BF16, tag="osb")
                if jc % 2 == 0:
                    nc.scalar.copy(o_sb[:], pO[:])
                else:
                    nc.vector.tensor_copy(o_sb[:], pO[:])
                nc.sync.dma_start(out[n, jc * P:(jc + 1) * P, :], o_sb[:])

        def body_all(_iv=None):
            run_prep()
            # Wt2 is first needed by H2 of graph 0; queueing its load after
            # the prep weights keeps the prep-critical DMAs in front.
            _dma_split(nc, Wt2_sb[:], Wt2.rearrange("(c p) d -> p c d", p=P), 2)
            for n in range(NG):
                phase2(n, phase1(n))

        if reps == 1:
            body_all()
        else:
            with tc.For_i(0, reps, 1) as _iv:
                body_all(_iv)
    return nc


_NC_CACHE = {}
TRACE = False
_LAST = {}


def _get_nc():
    if "nc" not in _NC_CACHE:
        nc = bacc.Bacc("TRN2", target_bir_lowering=False, debug=False)
        build(nc)
        nc.compile()
        _NC_CACHE["nc"] = nc
    return _NC_CACHE["nc"]


def kernel(input_state, adj, entity_mask, query_vec, W_type, a_type,
           qattn_W1, qattn_W2):
    from concourse import bass_utils
    nc = _get_nc()
    bf = ml_dtypes.bfloat16
    input_state = np.asarray(input_state, dtype=np.float32)
    adj = np.asarray(adj, dtype=np.int32)
    query_vec = np.asarray(query_vec, dtype=np.float32)

    xT_all = np.ascontiguousarray(
        input_state.transpose(0, 2, 1)).astype(bf)              # [N, K, E]
    masks_all = np.ascontiguousarray(np.stack(
        [(adj == 0), (adj == 2), (adj == 3)], axis=1)).astype(np.uint8)
    qT_all = np.ascontiguousarray(query_vec.T).astype(bf)       # [K, N]
    at_h = np.ascontiguousarray(
        np.asarray(a_type, np.float32).reshape(NT, DC2, P).transpose(2, 1, 0))
    WtT_h = np.ascontiguousarray(
        np.asarray(W_type, np.float32).transpose(0, 2, 1)).astype(bf)
    Wt2_h = np.ascontiguousarray(np.asarray(W_type, np.float32)[2]).astype(bf)
    W1_h = np.ascontiguousarray(np.asarray(qattn_W1, np.float32)).astype(bf)
    W2q_h = np.ascontiguousarray(np.asarray(qattn_W2, np.float32)).astype(bf)

    in_maps = []
    for c in range(N_CORES):
        sl = slice(c * NG, (c + 1) * NG)
        in_maps.append({
            "xT": xT_all[sl], "masks": masks_all[sl],
            "qT": np.ascontiguousarray(qT_all[:, sl]),
            "at": at_h, "WtT": WtT_h, "Wt2": Wt2_h,
            "W1": W1_h, "W2q": W2q_h,
        })
    res = bass_utils.run_bass_kernel_spmd(nc, in_maps, core_ids=list(range(N_CORES)),
                                          trace=TRACE, stitch_traces=TRACE)
    _LAST["exec_ns"] = res.exec_time_ns
    _LAST["mean_ns"] = res.mean_exec_time_ns
    _LAST["trace"] = res.instructions_and_trace
    _LAST["scope_times"] = res.per_core_scope_times
    out = np.concatenate([np.asarray(r["out"]) for r in res.results], axis=0)
    return out.astype(np.float32)


# revision 25
# speedup vs baseline: 1.7375x; 1.0269x over previous
"""GAT self-attention Trainium2 kernel (v2).

Full inputs -> shard graphs over 8 NeuronCores -> full output.

Math (per graph n, reference reformulated):
  g_i = sigmoid(relu(q @ W1_i) @ W2_i)            [2d]
  u_i^L = W_i @ (g_i[:d] * a_i[:d])               [k]   (left projector)
  u_i^R = W_i @ (g_i[d:] * a_i[d:])               [k]   (right projector)
  left_i = X @ u_i^L ; right_i = X @ u_i^R        [E]
  score[i,j] = lrelu(left_t[i] + right_t[j]), t = adj[i,j]; -BIG if adj==0
  Ex = exp(score); rs = rowsum(Ex)
  out = Ex^T @ (X @ W_2 / rs[:,None])             (== softmax(score)^T @ (X @ W_2))

Host staging: x/q/W transposed + bf16; adj one-hot masks as uint8.
Device layout trick: the LR matmul emits a 12-row block
  [L1, 1, L2, 1, L3, 1, 1, R1, 1, R2, 1, R3]
(ones rows filled by a rank-1 "needle" accumulation), so each type's
outer-sum score matmul reads its [L_t; 1] / [1; R_t] operand pair
directly -- no staging copies or DMAs.
"""
import numpy as np
from contextlib import ExitStack

import ml_dtypes

import concourse.bass as bass
import concourse.tile as tile
from concourse import mybir, bacc
from concourse.masks import make_identity

F32 = mybir.dt.float32
BF16 = mybir.dt.bfloat16
U8 = mybir.dt.uint8
AF = mybir.ActivationFunctionType
OP = mybir.AluOpType

N_CORES = 8
N, E, K, D = 64, 512, 512, 512   # graphs, entities, in_dim, out_dim
NG = N // N_CORES                # graphs per core
NT = 3                           # edge types
P = 128
EC = E // P                      # 4 partition chunks of E
KC = K // P
DC = D // P
TD2 = 2 * D
DC2 = TD2 // P                   # 8 chunks of the 2d gate dim
NEG_BIG = -200.0
LRELU_SLOPE = 0.2


def _dma_split(nc, dst, src, pieces):
    """Split a big load along dim 1 across sync/scalar queues."""
    n0 = dst.shape[1]
    step = max(1, n0 // pieces)
    engs = [nc.sync, nc.scalar]
    i = 0
    c = 0
    while i < n0:
        j = min(n0, i + step)
        engs[c % 2].dma_start(dst[:, i:j], src[:, i:j])
        i = j
        c += 1


def build(nc, reps=1):
    xT = nc.dram_tensor("xT", [NG, K, E], BF16, kind="ExternalInput").ap()
    masks = nc.dram_tensor("masks", [NG, 3, E, E], U8, kind="ExternalInput").ap()
    qT = nc.dram_tensor("qT", [K, NG], BF16, kind="ExternalInput").ap()
    at = nc.dram_tensor("at", [P, DC2, NT], F32, kind="ExternalInput").ap()
    WtT = nc.dram_tensor("WtT", [NT, D, K], BF16, kind="ExternalInput").ap()
    Wt2 = nc.dram_tensor("Wt2", [K, D], BF16, kind="ExternalInput").ap()
    W1 = nc.dram_tensor("W1", [NT, K, TD2], BF16, kind="ExternalInput").ap()
    W2q = nc.dram_tensor("W2q", [NT, TD2, TD2], BF16, kind="ExternalInput").ap()
    out = nc.dram_tensor("out", [NG, E, D], BF16, kind="ExternalOutput").ap()
    nc._gat_io = (xT, masks, qT, at, WtT, Wt2, W1, W2q, out)
    _build_once(nc, reps)


def _build_once(nc, reps=1):
    xT, masks, qT, at, WtT, Wt2, W1, W2q, out = nc._gat_io
    with tile.TileContext(nc) as tc, ExitStack() as ctx:
        # ---------------- persistent ----------------
        pers = ctx.enter_context(tc.tile_pool(name="pers", bufs=1))
        ident = pers.tile([P, P], F32)
        make_identity(nc, ident[:])
        ident_bf = pers.tile([P, P], BF16)
        nc.vector.tensor_copy(ident_bf[:], ident[:])
        # U_all[k%128, kc, c, n]: c=t -> u_t^L ; c=3+t -> u_t^R
        U_all = pers.tile([P, KC, 6, NG], BF16)
        qT_sb = pers.tile([P, KC, NG], BF16)
        nc.sync.dma_start(qT_sb[:], qT.rearrange("(c p) n -> p c n", p=P))
        at_sb = pers.tile([P, DC2, NT], F32)
        nc.sync.dma_start(at_sb[:], at)
        # Persistent AB ring: ones rows at fixed spots, L/R rows DMA'd per
        # graph. AB[32t + q, 0:E] = [L_t; 1], AB[32t + q, E:2E] = [1; R_t].
        AB_ring = []
        for r in range(3):
            ab = pers.tile([96, 2 * E], BF16, tag=f"ab{r}")
            for t in range(NT):
                nc.vector.memset(ab[32 * t + 1:32 * t + 2, 0:E], 1.0)
                nc.gpsimd.memset(ab[32 * t:32 * t + 1, E:2 * E], 1.0)
            AB_ring.append(ab)
        Wt2_sb = pers.tile([P, KC, D], BF16)

        # ---------------- pools ----------------
        sbuf = ctx.enter_context(tc.tile_pool(name="sbuf", bufs=3))
        perg = ctx.enter_context(tc.tile_pool(name="perg", bufs=NG))
        small = ctx.enter_context(tc.tile_pool(name="small", bufs=3))
        ps_v = ctx.enter_context(tc.tile_pool(name="ps_v", bufs=4, space="PSUM"))
        ps_big = ctx.enter_context(tc.tile_pool(name="ps_big", bufs=3, space="PSUM"))
        ps_lr = ctx.enter_context(tc.tile_pool(name="ps_lr", bufs=1, space="PSUM"))

        # ---------------- prep: gates -> U vectors ----------------
        # All gate matmuls use the NG(=8)-row operand as the 128-col-max
        # stationary side and stream the big weight as the moving side, so
        # each stage is a handful of ap=512 matmuls instead of dozens of
        # ap=8 ones; small [8, .] results are transposed back on the PE.
        # Emitted as a generator with a yield after each weight-bound stage
        # so prep-independent H2 blocks can be interleaved into the PE queue.
        prep = ctx.enter_context(tc.tile_pool(name="prep", bufs=1))

        def prep_type_stages(i):
            W1_sb = prep.tile([P, KC, TD2], BF16, tag="w1")
            src1 = W1[i].rearrange("(c p) f -> p c f", p=P)
            nc.sync.dma_start(W1_sb[:, :, 0:D], src1[:, :, 0:D])
            nc.scalar.dma_start(W1_sb[:, :, D:TD2], src1[:, :, D:TD2])
            # rr[n, o2] = relu(q @ W1_i), two 512-col halves
            rr_sb = prep.tile([NG, TD2], BF16, tag="rr")
            for h in range(2):
                pr = ps_big.tile([NG, D], F32, tag="big")
                for kc in range(KC):
                    nc.tensor.matmul(
                        pr[:], qT_sb[:, kc, :],
                        W1_sb[:, kc, h * D:(h + 1) * D],
                        start=(kc == 0), stop=(kc == KC - 1))
                nc.scalar.activation(rr_sb[:, h * D:(h + 1) * D], pr[:],
                                     AF.Relu)
            # rrT[o2%128, oc2, n] via PE transposes
            prT = ps_v.tile([P, DC2, NG], BF16, tag="v")
            for b in range(DC2):
                nc.tensor.transpose(prT[:, b, :],
                                    rr_sb[:, b * P:(b + 1) * P],
                                    ident_bf[:NG, :NG])
            rrT = prep.tile([P, DC2, NG], BF16, tag="rrT")
            nc.vector.tensor_copy(rrT[:], prT[:])
            yield
            W2_sb = prep.tile([P, DC2, TD2], BF16, tag="w2")
            src2 = W2q[i].rearrange("(c p) f -> p c f", p=P)
            for pc in range(4):
                dch, fh = pc % 2, pc // 2
                eng = nc.sync if pc % 2 == 0 else nc.scalar
                eng.dma_start(
                    W2_sb[:, dch * 4:(dch + 1) * 4, fh * D:(fh + 1) * D],
                    src2[:, dch * 4:(dch + 1) * 4, fh * D:(fh + 1) * D])
            # g[n, o2] = sigmoid(rr @ W2q_i)
            g_sb = prep.tile([NG, TD2], BF16, tag="g")
            for h in range(2):
                pg = ps_big.tile([NG, D], F32, tag="big")
                for dc in range(DC2):
                    nc.tensor.matmul(
                        pg[:], rrT[:, dc, :],
                        W2_sb[:, dc, h * D:(h + 1) * D],
                        start=(dc == 0), stop=(dc == DC2 - 1))
                nc.scalar.activation(g_sb[:, h * D:(h + 1) * D], pg[:],
                                     AF.Sigmoid)
            # vT[o2%128, dc, s, n] = g^T * a_i (a-mult fused into the
            # copy), (s, n) adjacent so both u-sides share one stationary
            pgT = ps_v.tile([P, DC2, NG], BF16, tag="v")
            for b in range(DC2):
                nc.tensor.transpose(pgT[:, b, :],
                                    g_sb[:, b * P:(b + 1) * P],
                                    ident_bf[:NG, :NG])
            vT = prep.tile([P, DC, 2, NG], BF16, tag="vT")
            for s in range(2):
                nc.vector.tensor_tensor(
                    vT[:, :, s, :], pgT[:, s * DC:(s + 1) * DC, :],
                    at_sb[:, s * DC:(s + 1) * DC, i:i + 1].broadcast_to(
                        (P, DC, NG)),
                    OP.mult)
            yield
            # u_i^{L,R}[n, k] = v-half @ W_i^T, both sides in one
            # 16-col stationary; transpose into U_all
            WtT_sb = prep.tile([P, DC, K], BF16, tag="wtt")
            _dma_split(nc, WtT_sb[:],
                       WtT[i].rearrange("(c p) k -> p c k", p=P), 2)
            pu = ps_big.tile([2 * NG, K], F32, tag="big")
            for dc in range(DC):
                nc.tensor.matmul(
                    pu[:], vT[:, dc, :, :], WtT_sb[:, dc, :],
                    start=(dc == 0), stop=(dc == DC - 1))
            u_sb = prep.tile([2 * NG, K], BF16, tag="u")
            nc.gpsimd.tensor_copy(u_sb[:], pu[:])
            puT = ps_v.tile([P, KC, 2 * NG], BF16, tag="v")
            for kc in range(KC):
                nc.tensor.transpose(puT[:, kc, :],
                                    u_sb[:, kc * P:(kc + 1) * P],
                                    ident_bf[:2 * NG, :2 * NG])
            nc.vector.tensor_copy(U_all[:, :, i, :], puT[:, :, 0:NG])
            nc.vector.tensor_copy(U_all[:, :, 3 + i, :], puT[:, :, NG:2 * NG])
            yield

        # ---------------- H2 = X @ W_2 (score-independent) ----------------
        xts = [None] * NG
        h2s = [None] * NG
        h2_engs = [nc.scalar, nc.vector, nc.gpsimd]

        def h2_one(n):
            Xt_sb = perg.tile([P, KC, E], BF16, tag="X")
            nc.sync.dma_start(Xt_sb[:], xT[n].rearrange("(c p) e -> p c e", p=P))
            H2_sb = perg.tile([P, EC, D], BF16, tag="H2")
            for ic in range(EC):
                pH = ps_big.tile([P, D], F32, tag="big")
                for kc in range(KC):
                    nc.tensor.matmul(pH[:], Xt_sb[:, kc, ic * P:(ic + 1) * P],
                                     Wt2_sb[:, kc, :],
                                     start=(kc == 0), stop=(kc == KC - 1))
                eng = h2_engs[(n * EC + ic) % 3]
                if eng is nc.scalar:
                    eng.copy(H2_sb[:, ic, :], pH[:])
                else:
                    eng.tensor_copy(H2_sb[:, ic, :], pH[:])
            xts[n] = Xt_sb
            h2s[n] = H2_sb

        # ---------------- main per-graph pipeline ----------------
        def phase1(n):
            """masks + the L/R rows + scattered outer-sum operands"""
            Xt_sb = xts[n]
            m_sb = sbuf.tile([P, 3, EC, E], U8, tag="m")
            nc.sync.dma_start(m_sb[:], masks[n].rearrange("m (c p) j -> p m c j", p=P))

            pLR = ps_lr.tile([6, E], F32, tag="lr")
            for kc in range(KC):
                nc.tensor.matmul(pLR[:], U_all[:, kc, :, n], Xt_sb[:, kc, :],
                                 start=(kc == 0), stop=(kc == KC - 1))
            LR_sb = small.tile([6, E], BF16, tag="lrs")
            nc.scalar.copy(LR_sb[:], pLR[:])
            # Scatter L/R rows to matmul-legal partition bases {0,32,64};
            # the ones rows are pre-set in the persistent ring tiles.
            AB = AB_ring[n % 3]
            ABg = AB.rearrange("(g q) e2 -> g q e2", q=32)
            nc.sync.dma_start(
                ABg[:, 0:1, 0:E],
                LR_sb[0:3].rearrange("(g q) e -> g q e", q=1))
            nc.sync.dma_start(
                ABg[:, 1:2, E:2 * E],
                LR_sb[3:6].rearrange("(g q) e -> g q e", q=1))
            return dict(Xt_sb=Xt_sb, m_sb=m_sb, AB=AB)

        def phase2(n, st):
            """scores, softmax stats, H2, output"""
            Xt_sb = st["Xt_sb"]; m_sb = st["m_sb"]; AB = st["AB"]
            E_sb = sbuf.tile([P, EC, E], BF16, tag="E")
            rs = small.tile([P, EC], F32, tag="rs")
            rsr = small.tile([P, EC], F32, tag="rsr")
            for ic in range(EC):
                pv = []
                for t in range(NT):
                    pvt = ps_v.tile([P, E], F32, tag="v")
                    nc.tensor.matmul(
                        pvt[:], AB[32 * t:32 * t + 2, ic * P:(ic + 1) * P],
                        AB[32 * t:32 * t + 2, E:2 * E], start=True, stop=True)
                    pv.append(pvt)
                nc.vector.copy_predicated(pv[0][:], m_sb[:, 1, ic, :], pv[1][:])
                nc.vector.copy_predicated(pv[0][:], m_sb[:, 2, ic, :], pv[2][:])
                # S += -BIG * m0 ; then lrelu(x) = max(0.2x, x)
                nc.gpsimd.scalar_tensor_tensor(
                    pv[0][:], m_sb[:, 0, ic, :], NEG_BIG, pv[0][:], OP.mult, OP.add)
                nc.gpsimd.scalar_tensor_tensor(
                    pv[0][:], pv[0][:], LRELU_SLOPE, pv[0][:], OP.mult, OP.max)
                nc.scalar.activation(E_sb[:, ic, :], pv[0][:], AF.Exp,
                                     accum_out=rs[:, ic:ic + 1])
                nc.vector.reciprocal(rsr[:, ic:ic + 1], rs[:, ic:ic + 1])
                # normalize Ex rows in place (bf16, cheap on DVE)
                nc.vector.tensor_scalar(E_sb[:, ic, :], E_sb[:, ic, :],
                                        rsr[:, ic:ic + 1], None, OP.mult)

            # out = Ex_norm^T @ H2
            H2s = h2s[n]
            for jc in range(EC):
                pO = ps_big.tile([P, D], F32, tag="big")
                for ic in range(EC):
                    nc.tensor.matmul(pO[:], E_sb[:, ic, jc * P:(jc + 1) * P],
                                     H2s[:, ic, :],
                                     start=(ic == 0), stop=(ic == EC - 1))
                o_sb = small.tile([P, D], BF16, tag="osb")
                if jc % 2 == 0:
                    nc.scalar.copy(o_sb[:], pO[:])
                else:
                    nc.vector.tensor_copy(o_sb[:], pO[:])
                nc.sync.dma_start(out[n, jc * P:(jc + 1) * P, :], o_sb[:])

        def body_all(_iv=None):
            # Interleave prep stages (weight-DMA bound) with H2 blocks
            # (prep-independent PE work) so the in-order PE queue never
            # stalls on a weight load.
            gens = [prep_type_stages(i) for i in range(NT)]
            next(gens[0])                   # W1[0] queued first
            _dma_split(nc, Wt2_sb[:], Wt2.rearrange("(c p) d -> p c d", p=P), 2)
            h2_one(0)
            next(gens[0]); h2_one(1)
            next(gens[0], None); h2_one(2)
            next(gens[1]); h2_one(3)
            next(gens[1]); h2_one(4)
            next(gens[1], None); h2_one(5)
            next(gens[2]); h2_one(6)
            next(gens[2]); h2_one(7)
            next(gens[2], None)
            for n in range(NG):
                phase2(n, phase1(n))

        if reps == 1:
            body_all()
        else:
            with tc.For_i(0, reps, 1) as _iv:
                body_all(_iv)
    return nc


_NC_CACHE = {}
TRACE = False
_LAST = {}


def _get_nc():
    if "nc" not in _NC_CACHE:
        nc = bacc.Bacc("TRN2", target_bir_lowering=False, debug=False)
        build(nc)
        nc.compile()
        _NC_CACHE["nc"] = nc
    return _NC_CACHE["nc"]


def kernel(input_state, adj, entity_mask, query_vec, W_type, a_type,
           qattn_W1, qattn_W2):
    from concourse import bass_utils
    nc = _get_nc()
    bf = ml_dtypes.bfloat16
    input_state = np.asarray(input_state, dtype=np.float32)
    adj = np.asarray(adj, dtype=np.int32)
    query_vec = np.asarray(query_vec, dtype=np.float32)

    xT_all = np.ascontiguousarray(
        input_state.transpose(0, 2, 1)).astype(bf)              # [N, K, E]
    masks_all = np.ascontiguousarray(np.stack(
        [(adj == 0), (adj == 2), (adj == 3)], axis=1)).astype(np.uint8)
    qT_all = np.ascontiguousarray(query_vec.T).astype(bf)       # [K, N]
    at_h = np.ascontiguousarray(
        np.asarray(a_type, np.float32).reshape(NT, DC2, P).transpose(2, 1, 0))
    WtT_h = np.ascontiguousarray(
        np.asarray(W_type, np.float32).transpose(0, 2, 1)).astype(bf)
    Wt2_h = np.ascontiguousarray(np.asarray(W_type, np.float32)[2]).astype(bf)
    W1_h = np.ascontiguousarray(np.asarray(qattn_W1, np.float32)).astype(bf)
    W2q_h = np.ascontiguousarray(np.asarray(qattn_W2, np.float32)).astype(bf)

    in_maps = []
    for c in range(N_CORES):
        sl = slice(c * NG, (c + 1) * NG)
        in_maps.append({
            "xT": xT_all[sl], "masks": masks_all[sl],
            "qT": np.ascontiguousarray(qT_all[:, sl]),
            "at": at_h, "WtT": WtT_h, "Wt2": Wt2_h,
            "W1": W1_h, "W2q": W2q_h,
        })
    res = bass_utils.run_bass_kernel_spmd(nc, in_maps, core_ids=list(range(N_CORES)),
                                          trace=TRACE, stitch_traces=TRACE)
    _LAST["exec_ns"] = res.exec_time_ns
    _LAST["mean_ns"] = res.mean_exec_time_ns
    _LAST["trace"] = res.instructions_and_trace
    _LAST["scope_times"] = res.per_core_scope_times
    out = np.concatenate([np.asarray(r["out"]) for r in res.results], axis=0)
    return out.astype(np.float32)


# revision 29
# speedup vs baseline: 1.7570x; 1.0112x over previous
"""GAT self-attention Trainium2 kernel (v2).

Full inputs -> shard graphs over 8 NeuronCores -> full output.

Math (per graph n, reference reformulated):
  g_i = sigmoid(relu(q @ W1_i) @ W2_i)            [2d]
  u_i^L = W_i @ (g_i[:d] * a_i[:d])               [k]   (left projector)
  u_i^R = W_i @ (g_i[d:] * a_i[d:])               [k]   (right projector)
  left_i = X @ u_i^L ; right_i = X @ u_i^R        [E]
  score[i,j] = lrelu(left_t[i] + right_t[j]), t = adj[i,j]; -BIG if adj==0
  Ex = exp(score); rs = rowsum(Ex)
  out = Ex^T @ (X @ W_2 / rs[:,None])             (== softmax(score)^T @ (X @ W_2))

Host staging: x/q/W transposed + bf16; adj one-hot masks as uint8.
Device layout trick: the LR matmul emits a 12-row block
  [L1, 1, L2, 1, L3, 1, 1, R1, 1, R2, 1, R3]
(ones rows filled by a rank-1 "needle" accumulation), so each type's
outer-sum score matmul reads its [L_t; 1] / [1; R_t] operand pair
directly -- no staging copies or DMAs.
"""
import numpy as np
from contextlib import ExitStack

import ml_dtypes

import concourse.bass as bass
import concourse.tile as tile
from concourse import mybir, bacc
from concourse.masks import make_identity

F32 = mybir.dt.float32
BF16 = mybir.dt.bfloat16
U8 = mybir.dt.uint8
AF = mybir.ActivationFunctionType
OP = mybir.AluOpType

N_CORES = 8
N, E, K, D = 64, 512, 512, 512   # graphs, entities, in_dim, out_dim
NG = N // N_CORES                # graphs per core
NT = 3                           # edge types
P = 128
EC = E // P                      # 4 partition chunks of E
KC = K // P
DC = D // P
TD2 = 2 * D
DC2 = TD2 // P                   # 8 chunks of the 2d gate dim
NEG_BIG = -200.0
LRELU_SLOPE = 0.2


def _dma_split(nc, dst, src, pieces):
    """Split a big load along dim 1 across sync/scalar queues."""
    n0 = dst.shape[1]
    step = max(1, n0 // pieces)
    engs = [nc.sync, nc.scalar]
    i = 0
    c = 0
    while i < n0:
        j = min(n0, i + step)
        engs[c % 2].dma_start(dst[:, i:j], src[:, i:j])
        i = j
        c += 1


def build(nc, reps=1):
    xT = nc.dram_tensor("xT", [NG, K, E], BF16, kind="ExternalInput").ap()
    masks = nc.dram_tensor("masks", [NG, 3, E, E], U8, kind="ExternalInput").ap()
    qT = nc.dram_tensor("qT", [K, NG], BF16, kind="ExternalInput").ap()
    at = nc.dram_tensor("at", [P, DC2, NT], F32, kind="ExternalInput").ap()
    WtT = nc.dram_tensor("WtT", [NT, D, K], BF16, kind="ExternalInput").ap()
    Wt2 = nc.dram_tensor("Wt2", [K, D], BF16, kind="ExternalInput").ap()
    W1 = nc.dram_tensor("W1", [NT, K, TD2], BF16, kind="ExternalInput").ap()
    W2q = nc.dram_tensor("W2q", [NT, TD2, TD2], BF16, kind="ExternalInput").ap()
    out = nc.dram_tensor("out", [NG, E, D], BF16, kind="ExternalOutput").ap()
    nc._gat_io = (xT, masks, qT, at, WtT, Wt2, W1, W2q, out)
    _build_once(nc, reps)


def _build_once(nc, reps=1):
    xT, masks, qT, at, WtT, Wt2, W1, W2q, out = nc._gat_io
    with tile.TileContext(nc) as tc, ExitStack() as ctx:
        # ---------------- persistent ----------------
        pers = ctx.enter_context(tc.tile_pool(name="pers", bufs=1))
        ident = pers.tile([P, P], F32)
        make_identity(nc, ident[:])
        ident_bf = pers.tile([P, P], BF16)
        nc.vector.tensor_copy(ident_bf[:], ident[:])
        # U_all[k%128, kc, c, n]: c=t -> u_t^L ; c=3+t -> u_t^R
        U_all = pers.tile([P, KC, 6, NG], BF16)
        qT_sb = pers.tile([P, KC, NG], BF16)
        nc.sync.dma_start(qT_sb[:], qT.rearrange("(c p) n -> p c n", p=P))
        at_sb = pers.tile([P, DC2, NT], F32)
        nc.sync.dma_start(at_sb[:], at)
        # Persistent AB ring: ones rows at fixed spots, L/R rows DMA'd per
        # graph. AB[32t + q, 0:E] = [L_t; 1], AB[32t + q, E:2E] = [1; R_t].
        AB_ring = []
        for r in range(4):
            ab = pers.tile([96, 2 * E], BF16, tag=f"ab{r}")
            for t in range(NT):
                nc.vector.memset(ab[32 * t + 1:32 * t + 2, 0:E], 1.0)
                nc.gpsimd.memset(ab[32 * t:32 * t + 1, E:2 * E], 1.0)
            AB_ring.append(ab)
        Wt2_sb = pers.tile([P, KC, D], BF16)

        # ---------------- pools ----------------
        sbuf = ctx.enter_context(tc.tile_pool(name="sbuf", bufs=3))
        perg = ctx.enter_context(tc.tile_pool(name="perg", bufs=NG))
        small = ctx.enter_context(tc.tile_pool(name="small", bufs=3))
        ps_v = ctx.enter_context(tc.tile_pool(name="ps_v", bufs=4, space="PSUM"))
        ps_big = ctx.enter_context(tc.tile_pool(name="ps_big", bufs=3, space="PSUM"))
        ps_lr = ctx.enter_context(tc.tile_pool(name="ps_lr", bufs=1, space="PSUM"))

        # ---------------- prep: gates -> U vectors ----------------
        # All gate matmuls use the NG(=8)-row operand as the 128-col-max
        # stationary side and stream the big weight as the moving side, so
        # each stage is a handful of ap=512 matmuls instead of dozens of
        # ap=8 ones; small [8, .] results are transposed back on the PE.
        # Emitted as a generator with a yield after each weight-bound stage
        # so prep-independent H2 blocks can be interleaved into the PE queue.
        prep = ctx.enter_context(tc.tile_pool(name="prep", bufs=1))

        def prep_type_stages(i):
            W1_sb = prep.tile([P, KC, TD2], BF16, tag="w1")
            src1 = W1[i].rearrange("(c p) f -> p c f", p=P)
            nc.sync.dma_start(W1_sb[:, :, 0:D], src1[:, :, 0:D])
            nc.scalar.dma_start(W1_sb[:, :, D:TD2], src1[:, :, D:TD2])
            # rr[n, o2] = relu(q @ W1_i), two 512-col halves
            rr_sb = prep.tile([NG, TD2], BF16, tag="rr")
            for h in range(2):
                pr = ps_big.tile([NG, D], F32, tag="big")
                for kc in range(KC):
                    nc.tensor.matmul(
                        pr[:], qT_sb[:, kc, :],
                        W1_sb[:, kc, h * D:(h + 1) * D],
                        start=(kc == 0), stop=(kc == KC - 1))
                nc.scalar.activation(rr_sb[:, h * D:(h + 1) * D], pr[:],
                                     AF.Relu)
            # rrT[o2%128, oc2, n] via PE transposes
            prT = ps_v.tile([P, DC2, NG], BF16, tag="v")
            for b in range(DC2):
                nc.tensor.transpose(prT[:, b, :],
                                    rr_sb[:, b * P:(b + 1) * P],
                                    ident_bf[:NG, :NG])
            rrT = prep.tile([P, DC2, NG], BF16, tag="rrT")
            nc.vector.tensor_copy(rrT[:], prT[:])
            yield
            W2_sb = prep.tile([P, DC2, TD2], BF16, tag="w2")
            src2 = W2q[i].rearrange("(c p) f -> p c f", p=P)
            for pc in range(4):
                dch, fh = pc % 2, pc // 2
                eng = nc.sync if pc % 2 == 0 else nc.scalar
                eng.dma_start(
                    W2_sb[:, dch * 4:(dch + 1) * 4, fh * D:(fh + 1) * D],
                    src2[:, dch * 4:(dch + 1) * 4, fh * D:(fh + 1) * D])
            # g[n, o2] = sigmoid(rr @ W2q_i)
            g_sb = prep.tile([NG, TD2], BF16, tag="g")
            for h in range(2):
                pg = ps_big.tile([NG, D], F32, tag="big")
                for dc in range(DC2):
                    nc.tensor.matmul(
                        pg[:], rrT[:, dc, :],
                        W2_sb[:, dc, h * D:(h + 1) * D],
                        start=(dc == 0), stop=(dc == DC2 - 1))
                nc.scalar.activation(g_sb[:, h * D:(h + 1) * D], pg[:],
                                     AF.Sigmoid)
            # vT[o2%128, dc, s, n] = g^T * a_i (a-mult fused into the
            # copy), (s, n) adjacent so both u-sides share one stationary
            pgT = ps_v.tile([P, DC2, NG], BF16, tag="v")
            for b in range(DC2):
                nc.tensor.transpose(pgT[:, b, :],
                                    g_sb[:, b * P:(b + 1) * P],
                                    ident_bf[:NG, :NG])
            vT = prep.tile([P, DC, 2, NG], BF16, tag="vT")
            for s in range(2):
                nc.vector.tensor_tensor(
                    vT[:, :, s, :], pgT[:, s * DC:(s + 1) * DC, :],
                    at_sb[:, s * DC:(s + 1) * DC, i:i + 1].broadcast_to(
                        (P, DC, NG)),
                    OP.mult)
            yield
            # u_i^{L,R}[n, k] = v-half @ W_i^T, both sides in one
            # 16-col stationary; transpose into U_all
            WtT_sb = prep.tile([P, DC, K], BF16, tag="wtt")
            _dma_split(nc, WtT_sb[:],
                       WtT[i].rearrange("(c p) k -> p c k", p=P), 2)
            pu = ps_big.tile([2 * NG, K], F32, tag="big")
            for dc in range(DC):
                nc.tensor.matmul(
                    pu[:], vT[:, dc, :, :], WtT_sb[:, dc, :],
                    start=(dc == 0), stop=(dc == DC - 1))
            u_sb = prep.tile([2 * NG, K], BF16, tag="u")
            nc.gpsimd.tensor_copy(u_sb[:], pu[:])
            puT = ps_v.tile([P, KC, 2 * NG], BF16, tag="v")
            for kc in range(KC):
                nc.tensor.transpose(puT[:, kc, :],
                                    u_sb[:, kc * P:(kc + 1) * P],
                                    ident_bf[:2 * NG, :2 * NG])
            nc.vector.tensor_copy(U_all[:, :, i, :], puT[:, :, 0:NG])
            nc.vector.tensor_copy(U_all[:, :, 3 + i, :], puT[:, :, NG:2 * NG])
            yield

        # ---------------- H2 = X @ W_2 (score-independent) ----------------
        xts = [None] * NG
        h2s = [None] * NG
        h2_engs = [nc.scalar, nc.vector, nc.gpsimd]

        def h2_one(n):
            Xt_sb = perg.tile([P, KC, E], BF16, tag="X")
            nc.sync.dma_start(Xt_sb[:], xT[n].rearrange("(c p) e -> p c e", p=P))
            H2_sb = perg.tile([P, EC, D], BF16, tag="H2")
            for ic in range(EC):
                pH = ps_big.tile([P, D], F32, tag="big")
                for kc in range(KC):
                    nc.tensor.matmul(pH[:], Xt_sb[:, kc, ic * P:(ic + 1) * P],
                                     Wt2_sb[:, kc, :],
                                     start=(kc == 0), stop=(kc == KC - 1))
                eng = h2_engs[(n * EC + ic) % 3]
                if eng is nc.scalar:
                    eng.copy(H2_sb[:, ic, :], pH[:])
                else:
                    eng.tensor_copy(H2_sb[:, ic, :], pH[:])
            xts[n] = Xt_sb
            h2s[n] = H2_sb

        # ---------------- main per-graph pipeline ----------------
        def phase1(n):
            """masks + the L/R rows + scattered outer-sum operands"""
            Xt_sb = xts[n]
            m_sb = sbuf.tile([P, 3, EC, E], U8, tag="m")
            nc.sync.dma_start(m_sb[:], masks[n].rearrange("m (c p) j -> p m c j", p=P))

            pLR = ps_lr.tile([6, E], F32, tag="lr")
            for kc in range(KC):
                nc.tensor.matmul(pLR[:], U_all[:, kc, :, n], Xt_sb[:, kc, :],
                                 start=(kc == 0), stop=(kc == KC - 1))
            LR_sb = small.tile([6, E], BF16, tag="lrs")
            nc.scalar.copy(LR_sb[:], pLR[:])
            # Scatter L/R rows to matmul-legal partition bases {0,32,64};
            # the ones rows are pre-set in the persistent ring tiles.
            AB = AB_ring[n % 4]
            ABg = AB.rearrange("(g q) e2 -> g q e2", q=32)
            nc.sync.dma_start(
                ABg[:, 0:1, 0:E],
                LR_sb[0:3].rearrange("(g q) e -> g q e", q=1))
            nc.sync.dma_start(
                ABg[:, 1:2, E:2 * E],
                LR_sb[3:6].rearrange("(g q) e -> g q e", q=1))
            return dict(Xt_sb=Xt_sb, m_sb=m_sb, AB=AB)

        def scores(n, st):
            """masked scores -> normalized exp matrix"""
            m_sb = st["m_sb"]; AB = st["AB"]
            E_sb = sbuf.tile([P, EC, E], BF16, tag="E")
            rs = small.tile([P, EC], F32, tag="rs")
            rsr = small.tile([P, EC], F32, tag="rsr")
            for ic in range(EC):
                pv = []
                for t in range(NT):
                    pvt = ps_v.tile([P, E], F32, tag="v")
                    nc.tensor.matmul(
                        pvt[:], AB[32 * t:32 * t + 2, ic * P:(ic + 1) * P],
                        AB[32 * t:32 * t + 2, E:2 * E], start=True, stop=True)
                    pv.append(pvt)
                nc.vector.copy_predicated(pv[0][:], m_sb[:, 1, ic, :], pv[1][:])
                nc.vector.copy_predicated(pv[0][:], m_sb[:, 2, ic, :], pv[2][:])
                # S += -BIG * m0 ; then lrelu(x) = max(0.2x, x)
                nc.gpsimd.scalar_tensor_tensor(
                    pv[0][:], m_sb[:, 0, ic, :], NEG_BIG, pv[0][:], OP.mult, OP.add)
                nc.gpsimd.scalar_tensor_tensor(
                    pv[0][:], pv[0][:], LRELU_SLOPE, pv[0][:], OP.mult, OP.max)
                nc.scalar.activation(E_sb[:, ic, :], pv[0][:], AF.Exp,
                                     accum_out=rs[:, ic:ic + 1])
                nc.vector.reciprocal(rsr[:, ic:ic + 1], rs[:, ic:ic + 1])
                # normalize Ex rows in place (bf16, cheap on DVE)
                nc.vector.tensor_scalar(E_sb[:, ic, :], E_sb[:, ic, :],
                                        rsr[:, ic:ic + 1], None, OP.mult)
            return E_sb

        def outp(n, E_sb):
            """out = Ex_norm^T @ H2"""
            H2s = h2s[n]
            for jc in range(EC):
                pO = ps_big.tile([P, D], F32, tag="big")
                for ic in range(EC):
                    nc.tensor.matmul(pO[:], E_sb[:, ic, jc * P:(jc + 1) * P],
                                     H2s[:, ic, :],
                                     start=(ic == 0), stop=(ic == EC - 1))
                o_sb = small.tile([P, D], BF16, tag="osb")
                if jc % 2 == 0:
                    nc.scalar.copy(o_sb[:], pO[:])
                else:
                    nc.vector.tensor_copy(o_sb[:], pO[:])
                nc.sync.dma_start(out[n, jc * P:(jc + 1) * P, :], o_sb[:])

        def body_all(_iv=None):
            # Interleave prep stages (weight-DMA bound) with H2 blocks
            # (prep-independent PE work) so the in-order PE queue never
            # stalls on a weight load.
            gens = [prep_type_stages(i) for i in range(NT)]
            next(gens[0])                   # W1[0] queued first
            _dma_split(nc, Wt2_sb[:], Wt2.rearrange("(c p) d -> p c d", p=P), 2)
            h2_one(0)
            next(gens[0]); h2_one(1)
            next(gens[0], None); h2_one(2)
            next(gens[1]); h2_one(3)
            next(gens[1]); h2_one(4)
            next(gens[1], None); h2_one(5)
            next(gens[2]); h2_one(6)
            next(gens[2]); h2_one(7)
            next(gens[2], None)
            # depth-2 software pipeline: LR/scores of graph n+2 are emitted
            # ahead of out(n) so the in-order PE queue always has independent
            # matmuls while the elementwise score chain of a graph drains.
            es = {0: scores(0, phase1(0)), 1: scores(1, phase1(1))}
            for n in range(NG):
                if n + 2 < NG:
                    es[n + 2] = scores(n + 2, phase1(n + 2))
                outp(n, es.pop(n))

        if reps == 1:
            body_all()
        else:
            with tc.For_i(0, reps, 1) as _iv:
                body_all(_iv)
    return nc


_NC_CACHE = {}
TRACE = False
_LAST = {}


def _get_nc():
    if "nc" not in _NC_CACHE:
        nc = bacc.Bacc("TRN2", target_bir_lowering=False, debug=False)
        build(nc)
        nc.compile()
        _NC_CACHE["nc"] = nc
    return _NC_CACHE["nc"]


def kernel(input_state, adj, entity_mask, query_vec, W_type, a_type,
           qattn_W1, qattn_W2):
    from concourse import bass_utils
    nc = _get_nc()
    bf = ml_dtypes.bfloat16
    input_state = np.asarray(input_state, dtype=np.float32)
    adj = np.asarray(adj, dtype=np.int32)
    query_vec = np.asarray(query_vec, dtype=np.float32)

    xT_all = np.ascontiguousarray(
        input_state.transpose(0, 2, 1)).astype(bf)              # [N, K, E]
    masks_all = np.ascontiguousarray(np.stack(
        [(adj == 0), (adj == 2), (adj == 3)], axis=1)).astype(np.uint8)
    qT_all = np.ascontiguousarray(query_vec.T).astype(bf)       # [K, N]
    at_h = np.ascontiguousarray(
        np.asarray(a_type, np.float32).reshape(NT, DC2, P).transpose(2, 1, 0))
    WtT_h = np.ascontiguousarray(
        np.asarray(W_type, np.float32).transpose(0, 2, 1)).astype(bf)
    Wt2_h = np.ascontiguousarray(np.asarray(W_type, np.float32)[2]).astype(bf)
    W1_h = np.ascontiguousarray(np.asarray(qattn_W1, np.float32)).astype(bf)
    W2q_h = np.ascontiguousarray(np.asarray(qattn_W2, np.float32)).astype(bf)

    in_maps = []
    for c in range(N_CORES):
        sl = slice(c * NG, (c + 1) * NG)
        in_maps.append({
            "xT": xT_all[sl], "masks": masks_all[sl],
            "qT": np.ascontiguousarray(qT_all[:, sl]),
            "at": at_h, "WtT": WtT_h, "Wt2": Wt2_h,
            "W1": W1_h, "W2q": W2q_h,
        })
    res = bass_utils.run_bass_kernel_spmd(nc, in_maps, core_ids=list(range(N_CORES)),
                                          trace=TRACE, stitch_traces=TRACE)
    _LAST["exec_ns"] = res.exec_time_ns
    _LAST["mean_ns"] = res.mean_exec_time_ns
    _LAST["trace"] = res.instructions_and_trace
    _LAST["scope_times"] = res.per_core_scope_times
    out = np.concatenate([np.asarray(r["out"]) for r in res.results], axis=0)
    return out.astype(np.float32)


# revision 30
# speedup vs baseline: 1.9186x; 1.0920x over previous
"""GAT self-attention Trainium2 kernel (v2).

Full inputs -> shard graphs over 8 NeuronCores -> full output.

Math (per graph n, reference reformulated):
  g_i = sigmoid(relu(q @ W1_i) @ W2_i)            [2d]
  u_i^L = W_i @ (g_i[:d] * a_i[:d])               [k]   (left projector)
  u_i^R = W_i @ (g_i[d:] * a_i[d:])               [k]   (right projector)
  left_i = X @ u_i^L ; right_i = X @ u_i^R        [E]
  score[i,j] = lrelu(left_t[i] + right_t[j]), t = adj[i,j]; -BIG if adj==0
  Ex = exp(score); rs = rowsum(Ex)
  out = Ex^T @ (X @ W_2 / rs[:,None])             (== softmax(score)^T @ (X @ W_2))

Host staging: x/q/W transposed + bf16; adj one-hot masks as uint8.
Device layout trick: the LR matmul emits a 12-row block
  [L1, 1, L2, 1, L3, 1, 1, R1, 1, R2, 1, R3]
(ones rows filled by a rank-1 "needle" accumulation), so each type's
outer-sum score matmul reads its [L_t; 1] / [1; R_t] operand pair
directly -- no staging copies or DMAs.
"""
import numpy as np
from contextlib import ExitStack

import ml_dtypes

import concourse.bass as bass
import concourse.tile as tile
from concourse import mybir, bacc
from concourse.masks import make_identity

F32 = mybir.dt.float32
BF16 = mybir.dt.bfloat16
U8 = mybir.dt.uint8
AF = mybir.ActivationFunctionType
OP = mybir.AluOpType

N_CORES = 8
N, E, K, D = 64, 512, 512, 512   # graphs, entities, in_dim, out_dim
NG = N // N_CORES                # graphs per core
NT = 3                           # edge types
P = 128
EC = E // P                      # 4 partition chunks of E
KC = K // P
DC = D // P
TD2 = 2 * D
DC2 = TD2 // P                   # 8 chunks of the 2d gate dim
NEG_BIG = -200.0
LRELU_SLOPE = 0.2


def _dma_split(nc, dst, src, pieces):
    """Split a big load along dim 1 across sync/scalar queues."""
    n0 = dst.shape[1]
    step = max(1, n0 // pieces)
    engs = [nc.sync, nc.scalar]
    i = 0
    c = 0
    while i < n0:
        j = min(n0, i + step)
        engs[c % 2].dma_start(dst[:, i:j], src[:, i:j])
        i = j
        c += 1


def build(nc, reps=1):
    xT = nc.dram_tensor("xT", [NG, K, E], BF16, kind="ExternalInput").ap()
    masks = nc.dram_tensor("masks", [NG, 3, E, E], U8, kind="ExternalInput").ap()
    qT = nc.dram_tensor("qT", [K, NG], BF16, kind="ExternalInput").ap()
    at = nc.dram_tensor("at", [P, DC2, NT], F32, kind="ExternalInput").ap()
    WtT = nc.dram_tensor("WtT", [NT, D, K], BF16, kind="ExternalInput").ap()
    Wt2 = nc.dram_tensor("Wt2", [K, D], BF16, kind="ExternalInput").ap()
    W1 = nc.dram_tensor("W1", [NT, K, TD2], BF16, kind="ExternalInput").ap()
    W2q = nc.dram_tensor("W2q", [NT, TD2, TD2], BF16, kind="ExternalInput").ap()
    out = nc.dram_tensor("out", [NG, E, D], BF16, kind="ExternalOutput").ap()
    nc._gat_io = (xT, masks, qT, at, WtT, Wt2, W1, W2q, out)
    _build_once(nc, reps)


def _build_once(nc, reps=1):
    xT, masks, qT, at, WtT, Wt2, W1, W2q, out = nc._gat_io
    with tile.TileContext(nc) as tc, ExitStack() as ctx:
        # ---------------- persistent ----------------
        pers = ctx.enter_context(tc.tile_pool(name="pers", bufs=1))
        ident = pers.tile([P, P], F32)
        make_identity(nc, ident[:])
        ident_bf = pers.tile([P, P], BF16)
        nc.vector.tensor_copy(ident_bf[:], ident[:])
        # U_all[k%128, kc, c, n]: c=t -> u_t^L ; c=3+t -> u_t^R
        U_all = pers.tile([P, KC, 6, NG], BF16)
        qT_sb = pers.tile([P, KC, NG], BF16)
        nc.sync.dma_start(qT_sb[:], qT.rearrange("(c p) n -> p c n", p=P))
        at_sb = pers.tile([P, DC2, NT], F32)
        nc.sync.dma_start(at_sb[:], at)
        # Persistent AB ring: ones rows at fixed spots, L/R rows DMA'd per
        # graph. AB[32t + q, 0:E] = [L_t; 1], AB[32t + q, E:2E] = [1; R_t].
        AB_ring = []
        for r in range(4):
            ab = pers.tile([96, 2 * E], BF16, tag=f"ab{r}")
            for t in range(NT):
                nc.vector.memset(ab[32 * t + 1:32 * t + 2, 0:E], 1.0)
                nc.gpsimd.memset(ab[32 * t:32 * t + 1, E:2 * E], 1.0)
            AB_ring.append(ab)
        Wt2_sb = pers.tile([P, KC, D], BF16)

        # ---------------- pools ----------------
        sbuf = ctx.enter_context(tc.tile_pool(name="sbuf", bufs=3))
        perg = ctx.enter_context(tc.tile_pool(name="perg", bufs=NG))
        small = ctx.enter_context(tc.tile_pool(name="small", bufs=3))
        ps_v = ctx.enter_context(tc.tile_pool(name="ps_v", bufs=5, space="PSUM"))
        ps_big = ctx.enter_context(tc.tile_pool(name="ps_big", bufs=2, space="PSUM"))
        ps_lr = ctx.enter_context(tc.tile_pool(name="ps_lr", bufs=1, space="PSUM"))

        # ---------------- prep: gates -> U vectors ----------------
        # All gate matmuls use the NG(=8)-row operand as the 128-col-max
        # stationary side and stream the big weight as the moving side, so
        # each stage is a handful of ap=512 matmuls instead of dozens of
        # ap=8 ones; small [8, .] results are transposed back on the PE.
        # Emitted as a generator with a yield after each weight-bound stage
        # so prep-independent H2 blocks can be interleaved into the PE queue.
        prep = ctx.enter_context(tc.tile_pool(name="prep", bufs=1))

        def prep_type_stages(i):
            W1_sb = prep.tile([P, KC, TD2], BF16, tag="w1")
            src1 = W1[i].rearrange("(c p) f -> p c f", p=P)
            nc.sync.dma_start(W1_sb[:, :, 0:D], src1[:, :, 0:D])
            nc.scalar.dma_start(W1_sb[:, :, D:TD2], src1[:, :, D:TD2])
            # rr[n, o2] = relu(q @ W1_i), two 512-col halves
            rr_sb = prep.tile([NG, TD2], BF16, tag="rr")
            for h in range(2):
                pr = ps_big.tile([NG, D], F32, tag="big")
                for kc in range(KC):
                    nc.tensor.matmul(
                        pr[:], qT_sb[:, kc, :],
                        W1_sb[:, kc, h * D:(h + 1) * D],
                        start=(kc == 0), stop=(kc == KC - 1))
                nc.scalar.activation(rr_sb[:, h * D:(h + 1) * D], pr[:],
                                     AF.Relu)
            # rrT[o2%128, oc2, n] via PE transposes
            prT = ps_v.tile([P, DC2, NG], BF16, tag="v")
            for b in range(DC2):
                nc.tensor.transpose(prT[:, b, :],
                                    rr_sb[:, b * P:(b + 1) * P],
                                    ident_bf[:NG, :NG])
            rrT = prep.tile([P, DC2, NG], BF16, tag="rrT")
            nc.vector.tensor_copy(rrT[:], prT[:])
            yield
            W2_sb = prep.tile([P, DC2, TD2], BF16, tag="w2")
            src2 = W2q[i].rearrange("(c p) f -> p c f", p=P)
            for pc in range(4):
                dch, fh = pc % 2, pc // 2
                eng = nc.sync if pc % 2 == 0 else nc.scalar
                eng.dma_start(
                    W2_sb[:, dch * 4:(dch + 1) * 4, fh * D:(fh + 1) * D],
                    src2[:, dch * 4:(dch + 1) * 4, fh * D:(fh + 1) * D])
            # g[n, o2] = sigmoid(rr @ W2q_i)
            g_sb = prep.tile([NG, TD2], BF16, tag="g")
            for h in range(2):
                pg = ps_big.tile([NG, D], F32, tag="big")
                for dc in range(DC2):
                    nc.tensor.matmul(
                        pg[:], rrT[:, dc, :],
                        W2_sb[:, dc, h * D:(h + 1) * D],
                        start=(dc == 0), stop=(dc == DC2 - 1))
                nc.scalar.activation(g_sb[:, h * D:(h + 1) * D], pg[:],
                                     AF.Sigmoid)
            # vT[o2%128, dc, s, n] = g^T * a_i (a-mult fused into the
            # copy), (s, n) adjacent so both u-sides share one stationary
            pgT = ps_v.tile([P, DC2, NG], BF16, tag="v")
            for b in range(DC2):
                nc.tensor.transpose(pgT[:, b, :],
                                    g_sb[:, b * P:(b + 1) * P],
                                    ident_bf[:NG, :NG])
            vT = prep.tile([P, DC, 2, NG], BF16, tag="vT")
            for s in range(2):
                nc.vector.tensor_tensor(
                    vT[:, :, s, :], pgT[:, s * DC:(s + 1) * DC, :],
                    at_sb[:, s * DC:(s + 1) * DC, i:i + 1].broadcast_to(
                        (P, DC, NG)),
                    OP.mult)
            yield
            # u_i^{L,R}[n, k] = v-half @ W_i^T, both sides in one
            # 16-col stationary; transpose into U_all
            WtT_sb = prep.tile([P, DC, K], BF16, tag="wtt")
            _dma_split(nc, WtT_sb[:],
                       WtT[i].rearrange("(c p) k -> p c k", p=P), 2)
            pu = ps_big.tile([2 * NG, K], F32, tag="big")
            for dc in range(DC):
                nc.tensor.matmul(
                    pu[:], vT[:, dc, :, :], WtT_sb[:, dc, :],
                    start=(dc == 0), stop=(dc == DC - 1))
            u_sb = prep.tile([2 * NG, K], BF16, tag="u")
            nc.gpsimd.tensor_copy(u_sb[:], pu[:])
            puT = ps_v.tile([P, KC, 2 * NG], BF16, tag="v")
            for kc in range(KC):
                nc.tensor.transpose(puT[:, kc, :],
                                    u_sb[:, kc * P:(kc + 1) * P],
                                    ident_bf[:2 * NG, :2 * NG])
            nc.vector.tensor_copy(U_all[:, :, i, :], puT[:, :, 0:NG])
            nc.vector.tensor_copy(U_all[:, :, 3 + i, :], puT[:, :, NG:2 * NG])
            yield

        # ---------------- H2 = X @ W_2 (score-independent) ----------------
        xts = [None] * NG
        h2s = [None] * NG
        h2_engs = [nc.scalar, nc.vector, nc.gpsimd]

        def h2_one(n):
            Xt_sb = perg.tile([P, KC, E], BF16, tag="X")
            nc.sync.dma_start(Xt_sb[:], xT[n].rearrange("(c p) e -> p c e", p=P))
            H2_sb = perg.tile([P, EC, D], BF16, tag="H2")
            for ic in range(EC):
                pH = ps_big.tile([P, D], F32, tag="big")
                for kc in range(KC):
                    nc.tensor.matmul(pH[:], Xt_sb[:, kc, ic * P:(ic + 1) * P],
                                     Wt2_sb[:, kc, :],
                                     start=(kc == 0), stop=(kc == KC - 1))
                eng = h2_engs[(n * EC + ic) % 3]
                if eng is nc.scalar:
                    eng.copy(H2_sb[:, ic, :], pH[:])
                else:
                    eng.tensor_copy(H2_sb[:, ic, :], pH[:])
            xts[n] = Xt_sb
            h2s[n] = H2_sb

        # ---------------- main per-graph pipeline ----------------
        def phase1(n):
            """masks + the L/R rows + scattered outer-sum operands"""
            Xt_sb = xts[n]
            m_sb = sbuf.tile([P, 3, EC, E], U8, tag="m")
            nc.sync.dma_start(m_sb[:], masks[n].rearrange("m (c p) j -> p m c j", p=P))

            pLR = ps_lr.tile([6, E], F32, tag="lr")
            for kc in range(KC):
                nc.tensor.matmul(pLR[:], U_all[:, kc, :, n], Xt_sb[:, kc, :],
                                 start=(kc == 0), stop=(kc == KC - 1))
            LR_sb = small.tile([6, E], BF16, tag="lrs")
            nc.scalar.copy(LR_sb[:], pLR[:])
            # Scatter L/R rows to matmul-legal partition bases {0,32,64};
            # the ones rows are pre-set in the persistent ring tiles.
            AB = AB_ring[n % 4]
            ABg = AB.rearrange("(g q) e2 -> g q e2", q=32)
            nc.sync.dma_start(
                ABg[:, 0:1, 0:E],
                LR_sb[0:3].rearrange("(g q) e -> g q e", q=1))
            nc.sync.dma_start(
                ABg[:, 1:2, E:2 * E],
                LR_sb[3:6].rearrange("(g q) e -> g q e", q=1))
            return dict(Xt_sb=Xt_sb, m_sb=m_sb, AB=AB)

        def scores(n, st):
            """masked scores -> normalized exp matrix"""
            m_sb = st["m_sb"]; AB = st["AB"]
            E_sb = sbuf.tile([P, EC, E], BF16, tag="E")
            rs = small.tile([P, EC], F32, tag="rs")
            rsr = small.tile([P, EC], F32, tag="rsr")
            for ic in range(EC):
                pv = []
                for t in range(NT):
                    pvt = ps_v.tile([P, E], F32, tag="v")
                    nc.tensor.matmul(
                        pvt[:], AB[32 * t:32 * t + 2, ic * P:(ic + 1) * P],
                        AB[32 * t:32 * t + 2, E:2 * E], start=True, stop=True)
                    pv.append(pvt)
                nc.vector.copy_predicated(pv[0][:], m_sb[:, 1, ic, :], pv[1][:])
                nc.vector.copy_predicated(pv[0][:], m_sb[:, 2, ic, :], pv[2][:])
                # S += -BIG * m0 ; then lrelu(x) = max(0.2x, x)
                nc.gpsimd.scalar_tensor_tensor(
                    pv[0][:], m_sb[:, 0, ic, :], NEG_BIG, pv[0][:], OP.mult, OP.add)
                nc.gpsimd.scalar_tensor_tensor(
                    pv[0][:], pv[0][:], LRELU_SLOPE, pv[0][:], OP.mult, OP.max)
                nc.scalar.activation(E_sb[:, ic, :], pv[0][:], AF.Exp,
                                     accum_out=rs[:, ic:ic + 1])
                nc.vector.reciprocal(rsr[:, ic:ic + 1], rs[:, ic:ic + 1])
                # normalize Ex rows in place (bf16, cheap on DVE)
                nc.vector.tensor_scalar(E_sb[:, ic, :], E_sb[:, ic, :],
                                        rsr[:, ic:ic + 1], None, OP.mult)
            return E_sb

        def outp(n, E_sb):
            """out = Ex_norm^T @ H2"""
            H2s = h2s[n]
            for jc in range(EC):
                pO = ps_big.tile([P, D], F32, tag="big")
                for ic in range(EC):
                    nc.tensor.matmul(pO[:], E_sb[:, ic, jc * P:(jc + 1) * P],
                                     H2s[:, ic, :],
                                     start=(ic == 0), stop=(ic == EC - 1))
                o_sb = small.tile([P, D], BF16, tag="osb")
                nc.scalar.copy(o_sb[:], pO[:])
                nc.sync.dma_start(out[n, jc * P:(jc + 1) * P, :], o_sb[:])

        def body_all(_iv=None):
            # Interleave prep stages (weight-DMA bound) with H2 blocks
            # (prep-independent PE work) so the in-order PE queue never
            # stalls on a weight load.
            gens = [prep_type_stages(i) for i in range(NT)]
            next(gens[0])                   # W1[0] queued first
            _dma_split(nc, Wt2_sb[:], Wt2.rearrange("(c p) d -> p c d", p=P), 2)
            h2_one(0)
            next(gens[0]); h2_one(1)
            next(gens[0], None); h2_one(2)
            next(gens[1]); h2_one(3)
            next(gens[1]); h2_one(4)
            next(gens[1], None); h2_one(5)
            next(gens[2]); h2_one(6)
            next(gens[2]); h2_one(7)
            next(gens[2], None)
            # depth-2 software pipeline: LR/scores of graph n+2 are emitted
            # ahead of out(n) so the in-order PE queue always has independent
            # matmuls while the elementwise score chain of a graph drains.
            es = {0: scores(0, phase1(0)), 1: scores(1, phase1(1))}
            for n in range(NG):
                if n + 2 < NG:
                    es[n + 2] = scores(n + 2, phase1(n + 2))
                outp(n, es.pop(n))

        if reps == 1:
            body_all()
        else:
            with tc.For_i(0, reps, 1) as _iv:
                body_all(_iv)
    return nc


_NC_CACHE = {}
TRACE = False
_LAST = {}


def _get_nc():
    if "nc" not in _NC_CACHE:
        nc = bacc.Bacc("TRN2", target_bir_lowering=False, debug=False)
        build(nc)
        nc.compile()
        _NC_CACHE["nc"] = nc
    return _NC_CACHE["nc"]


def kernel(input_state, adj, entity_mask, query_vec, W_type, a_type,
           qattn_W1, qattn_W2):
    from concourse import bass_utils
    nc = _get_nc()
    bf = ml_dtypes.bfloat16
    input_state = np.asarray(input_state, dtype=np.float32)
    adj = np.asarray(adj, dtype=np.int32)
    query_vec = np.asarray(query_vec, dtype=np.float32)

    xT_all = np.ascontiguousarray(
        input_state.transpose(0, 2, 1)).astype(bf)              # [N, K, E]
    masks_all = np.ascontiguousarray(np.stack(
        [(adj == 0), (adj == 2), (adj == 3)], axis=1)).astype(np.uint8)
    qT_all = np.ascontiguousarray(query_vec.T).astype(bf)       # [K, N]
    at_h = np.ascontiguousarray(
        np.asarray(a_type, np.float32).reshape(NT, DC2, P).transpose(2, 1, 0))
    WtT_h = np.ascontiguousarray(
        np.asarray(W_type, np.float32).transpose(0, 2, 1)).astype(bf)
    Wt2_h = np.ascontiguousarray(np.asarray(W_type, np.float32)[2]).astype(bf)
    W1_h = np.ascontiguousarray(np.asarray(qattn_W1, np.float32)).astype(bf)
    W2q_h = np.ascontiguousarray(np.asarray(qattn_W2, np.float32)).astype(bf)

    in_maps = []
    for c in range(N_CORES):
        sl = slice(c * NG, (c + 1) * NG)
        in_maps.append({
            "xT": xT_all[sl], "masks": masks_all[sl],
            "qT": np.ascontiguousarray(qT_all[:, sl]),
            "at": at_h, "WtT": WtT_h, "Wt2": Wt2_h,
            "W1": W1_h, "W2q": W2q_h,
        })
    res = bass_utils.run_bass_kernel_spmd(nc, in_maps, core_ids=list(range(N_CORES)),
                                          trace=TRACE, stitch_traces=TRACE)
    _LAST["exec_ns"] = res.exec_time_ns
    _LAST["mean_ns"] = res.mean_exec_time_ns
    _LAST["trace"] = res.instructions_and_trace
    _LAST["scope_times"] = res.per_core_scope_times
    out = np.concatenate([np.asarray(r["out"]) for r in res.results], axis=0)
    return out.astype(np.float32)


# revision 32
# speedup vs baseline: 1.9607x; 1.0219x over previous
"""GAT self-attention Trainium2 kernel (v2).

Full inputs -> shard graphs over 8 NeuronCores -> full output.

Math (per graph n, reference reformulated):
  g_i = sigmoid(relu(q @ W1_i) @ W2_i)            [2d]
  u_i^L = W_i @ (g_i[:d] * a_i[:d])               [k]   (left projector)
  u_i^R = W_i @ (g_i[d:] * a_i[d:])               [k]   (right projector)
  left_i = X @ u_i^L ; right_i = X @ u_i^R        [E]
  score[i,j] = lrelu(left_t[i] + right_t[j]), t = adj[i,j]; -BIG if adj==0
  Ex = exp(score); rs = rowsum(Ex)
  out = Ex^T @ (X @ W_2 / rs[:,None])             (== softmax(score)^T @ (X @ W_2))

Host staging: x/q/W transposed + bf16; adj one-hot masks as uint8.
Device layout trick: the LR matmul emits a 12-row block
  [L1, 1, L2, 1, L3, 1, 1, R1, 1, R2, 1, R3]
(ones rows filled by a rank-1 "needle" accumulation), so each type's
outer-sum score matmul reads its [L_t; 1] / [1; R_t] operand pair
directly -- no staging copies or DMAs.
"""
import numpy as np
from contextlib import ExitStack

import ml_dtypes

import concourse.bass as bass
import concourse.tile as tile
from concourse import mybir, bacc
from concourse.masks import make_identity

F32 = mybir.dt.float32
BF16 = mybir.dt.bfloat16
U8 = mybir.dt.uint8
AF = mybir.ActivationFunctionType
OP = mybir.AluOpType

N_CORES = 8
N, E, K, D = 64, 512, 512, 512   # graphs, entities, in_dim, out_dim
NG = N // N_CORES                # graphs per core
NT = 3                           # edge types
P = 128
EC = E // P                      # 4 partition chunks of E
KC = K // P
DC = D // P
TD2 = 2 * D
DC2 = TD2 // P                   # 8 chunks of the 2d gate dim
NEG_BIG = -200.0
LRELU_SLOPE = 0.2


def _dma_split(nc, dst, src, pieces):
    """Split a big load along dim 1 across sync/scalar queues."""
    n0 = dst.shape[1]
    step = max(1, n0 // pieces)
    engs = [nc.sync, nc.scalar]
    i = 0
    c = 0
    while i < n0:
        j = min(n0, i + step)
        engs[c % 2].dma_start(dst[:, i:j], src[:, i:j])
        i = j
        c += 1


def build(nc, reps=1):
    xT = nc.dram_tensor("xT", [NG, K, E], BF16, kind="ExternalInput").ap()
    masks = nc.dram_tensor("masks", [NG, 3, E, E], U8, kind="ExternalInput").ap()
    qT = nc.dram_tensor("qT", [K, NG], BF16, kind="ExternalInput").ap()
    at = nc.dram_tensor("at", [P, DC2, NT], F32, kind="ExternalInput").ap()
    WtT = nc.dram_tensor("WtT", [NT, D, K], BF16, kind="ExternalInput").ap()
    Wt2 = nc.dram_tensor("Wt2", [K, D], BF16, kind="ExternalInput").ap()
    W1 = nc.dram_tensor("W1", [NT, K, TD2], BF16, kind="ExternalInput").ap()
    W2q = nc.dram_tensor("W2q", [NT, TD2, TD2], BF16, kind="ExternalInput").ap()
    out = nc.dram_tensor("out", [NG, E, D], BF16, kind="ExternalOutput").ap()
    nc._gat_io = (xT, masks, qT, at, WtT, Wt2, W1, W2q, out)
    _build_once(nc, reps)


def _build_once(nc, reps=1):
    xT, masks, qT, at, WtT, Wt2, W1, W2q, out = nc._gat_io
    with tile.TileContext(nc) as tc, ExitStack() as ctx:
        # ---------------- persistent ----------------
        pers = ctx.enter_context(tc.tile_pool(name="pers", bufs=1))
        ident = pers.tile([P, P], F32)
        make_identity(nc, ident[:])
        ident_bf = pers.tile([P, P], BF16)
        nc.vector.tensor_copy(ident_bf[:], ident[:])
        # U_all[k%128, kc, c, n]: c=t -> u_t^L ; c=3+t -> u_t^R
        U_all = pers.tile([P, KC, 6, NG], BF16)
        qT_sb = pers.tile([P, KC, NG], BF16)
        nc.gpsimd.dma_start(qT_sb[:], qT.rearrange("(c p) n -> p c n", p=P))
        at_sb = pers.tile([P, DC2, NT], F32)
        nc.gpsimd.dma_start(at_sb[:], at)
        # Persistent AB ring: ones rows at fixed spots, L/R rows DMA'd per
        # graph. AB[32t + q, 0:E] = [L_t; 1], AB[32t + q, E:2E] = [1; R_t].
        AB_ring = []
        for r in range(4):
            ab = pers.tile([96, 2 * E], BF16, tag=f"ab{r}")
            for t in range(NT):
                nc.vector.memset(ab[32 * t + 1:32 * t + 2, 0:E], 1.0)
                nc.gpsimd.memset(ab[32 * t:32 * t + 1, E:2 * E], 1.0)
            AB_ring.append(ab)
        Wt2_sb = pers.tile([P, KC, D], BF16)

        # ---------------- pools ----------------
        sbuf = ctx.enter_context(tc.tile_pool(name="sbuf", bufs=3))
        perg = ctx.enter_context(tc.tile_pool(name="perg", bufs=NG))
        small = ctx.enter_context(tc.tile_pool(name="small", bufs=3))
        ps_v = ctx.enter_context(tc.tile_pool(name="ps_v", bufs=5, space="PSUM"))
        ps_big = ctx.enter_context(tc.tile_pool(name="ps_big", bufs=2, space="PSUM"))
        ps_lr = ctx.enter_context(tc.tile_pool(name="ps_lr", bufs=1, space="PSUM"))

        # ---------------- prep: gates -> U vectors ----------------
        # All gate matmuls use the NG(=8)-row operand as the 128-col-max
        # stationary side and stream the big weight as the moving side, so
        # each stage is a handful of ap=512 matmuls instead of dozens of
        # ap=8 ones; small [8, .] results are transposed back on the PE.
        # Emitted as a generator with a yield after each weight-bound stage
        # so prep-independent H2 blocks can be interleaved into the PE queue.
        prep = ctx.enter_context(tc.tile_pool(name="prep", bufs=1))

        def prep_type_stages(i):
            W1_sb = prep.tile([P, KC, TD2], BF16, tag="w1")
            src1 = W1[i].rearrange("(c p) f -> p c f", p=P)
            nc.sync.dma_start(W1_sb[:, :, 0:D], src1[:, :, 0:D])
            nc.scalar.dma_start(W1_sb[:, :, D:TD2], src1[:, :, D:TD2])
            # rr[n, o2] = relu(q @ W1_i), two 512-col halves
            rr_sb = prep.tile([NG, TD2], BF16, tag="rr")
            for h in range(2):
                pr = ps_big.tile([NG, D], F32, tag="big")
                for kc in range(KC):
                    nc.tensor.matmul(
                        pr[:], qT_sb[:, kc, :],
                        W1_sb[:, kc, h * D:(h + 1) * D],
                        start=(kc == 0), stop=(kc == KC - 1))
                nc.scalar.activation(rr_sb[:, h * D:(h + 1) * D], pr[:],
                                     AF.Relu)
            # rrT[o2%128, oc2, n] via PE transposes
            prT = ps_v.tile([P, DC2, NG], BF16, tag="v")
            for b in range(DC2):
                nc.tensor.transpose(prT[:, b, :],
                                    rr_sb[:, b * P:(b + 1) * P],
                                    ident_bf[:NG, :NG])
            rrT = prep.tile([P, DC2, NG], BF16, tag="rrT")
            nc.vector.tensor_copy(rrT[:], prT[:])
            yield
            W2_sb = prep.tile([P, DC2, TD2], BF16, tag="w2")
            src2 = W2q[i].rearrange("(c p) f -> p c f", p=P)
            for pc in range(4):
                dch, fh = pc % 2, pc // 2
                eng = nc.sync if pc % 2 == 0 else nc.scalar
                eng.dma_start(
                    W2_sb[:, dch * 4:(dch + 1) * 4, fh * D:(fh + 1) * D],
                    src2[:, dch * 4:(dch + 1) * 4, fh * D:(fh + 1) * D])
            # g[n, o2] = sigmoid(rr @ W2q_i)
            g_sb = prep.tile([NG, TD2], BF16, tag="g")
            for h in range(2):
                pg = ps_big.tile([NG, D], F32, tag="big")
                for dc in range(DC2):
                    nc.tensor.matmul(
                        pg[:], rrT[:, dc, :],
                        W2_sb[:, dc, h * D:(h + 1) * D],
                        start=(dc == 0), stop=(dc == DC2 - 1))
                nc.scalar.activation(g_sb[:, h * D:(h + 1) * D], pg[:],
                                     AF.Sigmoid)
            # vT[o2%128, dc, s, n] = g^T * a_i (a-mult fused into the
            # copy), (s, n) adjacent so both u-sides share one stationary
            pgT = ps_v.tile([P, DC2, NG], BF16, tag="v")
            for b in range(DC2):
                nc.tensor.transpose(pgT[:, b, :],
                                    g_sb[:, b * P:(b + 1) * P],
                                    ident_bf[:NG, :NG])
            vT = prep.tile([P, DC, 2, NG], BF16, tag="vT")
            for s in range(2):
                nc.vector.tensor_tensor(
                    vT[:, :, s, :], pgT[:, s * DC:(s + 1) * DC, :],
                    at_sb[:, s * DC:(s + 1) * DC, i:i + 1].broadcast_to(
                        (P, DC, NG)),
                    OP.mult)
            yield
            # u_i^{L,R}[n, k] = v-half @ W_i^T, both sides in one
            # 16-col stationary; transpose into U_all
            WtT_sb = prep.tile([P, DC, K], BF16, tag="wtt")
            _dma_split(nc, WtT_sb[:],
                       WtT[i].rearrange("(c p) k -> p c k", p=P), 2)
            pu = ps_big.tile([2 * NG, K], F32, tag="big")
            for dc in range(DC):
                nc.tensor.matmul(
                    pu[:], vT[:, dc, :, :], WtT_sb[:, dc, :],
                    start=(dc == 0), stop=(dc == DC - 1))
            u_sb = prep.tile([2 * NG, K], BF16, tag="u")
            nc.gpsimd.tensor_copy(u_sb[:], pu[:])
            puT = ps_v.tile([P, KC, 2 * NG], BF16, tag="v")
            for kc in range(KC):
                nc.tensor.transpose(puT[:, kc, :],
                                    u_sb[:, kc * P:(kc + 1) * P],
                                    ident_bf[:2 * NG, :2 * NG])
            nc.vector.tensor_copy(U_all[:, :, i, :], puT[:, :, 0:NG])
            nc.vector.tensor_copy(U_all[:, :, 3 + i, :], puT[:, :, NG:2 * NG])
            yield

        # ---------------- H2 = X @ W_2 (score-independent) ----------------
        xts = [None] * NG
        h2s = [None] * NG
        h2_engs = [nc.scalar, nc.vector, nc.gpsimd]

        def h2_one(n):
            Xt_sb = perg.tile([P, KC, E], BF16, tag="X")
            nc.sync.dma_start(Xt_sb[:], xT[n].rearrange("(c p) e -> p c e", p=P))
            H2_sb = perg.tile([P, EC, D], BF16, tag="H2")
            for ic in range(EC):
                pH = ps_big.tile([P, D], F32, tag="big")
                for kc in range(KC):
                    nc.tensor.matmul(pH[:], Xt_sb[:, kc, ic * P:(ic + 1) * P],
                                     Wt2_sb[:, kc, :],
                                     start=(kc == 0), stop=(kc == KC - 1))
                eng = h2_engs[(n * EC + ic) % 3]
                if eng is nc.scalar:
                    eng.copy(H2_sb[:, ic, :], pH[:])
                else:
                    eng.tensor_copy(H2_sb[:, ic, :], pH[:])
            xts[n] = Xt_sb
            h2s[n] = H2_sb

        # ---------------- main per-graph pipeline ----------------
        def phase1(n):
            """masks + the L/R rows + scattered outer-sum operands"""
            Xt_sb = xts[n]
            m_sb = sbuf.tile([P, 3, EC, E], U8, tag="m")
            nc.sync.dma_start(m_sb[:], masks[n].rearrange("m (c p) j -> p m c j", p=P))

            pLR = ps_lr.tile([6, E], F32, tag="lr")
            for kc in range(KC):
                nc.tensor.matmul(pLR[:], U_all[:, kc, :, n], Xt_sb[:, kc, :],
                                 start=(kc == 0), stop=(kc == KC - 1))
            LR_sb = small.tile([6, E], BF16, tag="lrs")
            nc.scalar.copy(LR_sb[:], pLR[:])
            # Scatter L/R rows to matmul-legal partition bases {0,32,64};
            # the ones rows are pre-set in the persistent ring tiles.
            AB = AB_ring[n % 4]
            ABg = AB.rearrange("(g q) e2 -> g q e2", q=32)
            nc.sync.dma_start(
                ABg[:, 0:1, 0:E],
                LR_sb[0:3].rearrange("(g q) e -> g q e", q=1))
            nc.sync.dma_start(
                ABg[:, 1:2, E:2 * E],
                LR_sb[3:6].rearrange("(g q) e -> g q e", q=1))
            return dict(Xt_sb=Xt_sb, m_sb=m_sb, AB=AB)

        def scores(n, st):
            """masked scores -> normalized exp matrix"""
            m_sb = st["m_sb"]; AB = st["AB"]
            E_sb = sbuf.tile([P, EC, E], BF16, tag="E")
            rs = small.tile([P, EC], F32, tag="rs")
            rsr = small.tile([P, EC], F32, tag="rsr")
            for ic in range(EC):
                # merge tree: S3 into S2's bank first (frees it early), then
                # into S1 via the combined m2|m3 mask -- peak 2 live banks.
                pva = ps_v.tile([P, E], F32, tag="v")
                nc.tensor.matmul(
                    pva[:], AB[32:34, ic * P:(ic + 1) * P],
                    AB[32:34, E:2 * E], start=True, stop=True)
                pvb = ps_v.tile([P, E], F32, tag="v")
                nc.tensor.matmul(
                    pvb[:], AB[64:66, ic * P:(ic + 1) * P],
                    AB[64:66, E:2 * E], start=True, stop=True)
                nc.vector.copy_predicated(pva[:], m_sb[:, 2, ic, :], pvb[:])
                pvc = ps_v.tile([P, E], F32, tag="v")
                nc.tensor.matmul(
                    pvc[:], AB[0:2, ic * P:(ic + 1) * P],
                    AB[0:2, E:2 * E], start=True, stop=True)
                nc.vector.copy_predicated(pvc[:], m_sb[:, 1, ic, :], pva[:])
                # S += -BIG * m0 ; then lrelu(x) = max(0.2x, x)
                nc.gpsimd.scalar_tensor_tensor(
                    pvc[:], m_sb[:, 0, ic, :], NEG_BIG, pvc[:], OP.mult, OP.add)
                nc.gpsimd.scalar_tensor_tensor(
                    pvc[:], pvc[:], LRELU_SLOPE, pvc[:], OP.mult, OP.max)
                nc.scalar.activation(E_sb[:, ic, :], pvc[:], AF.Exp,
                                     accum_out=rs[:, ic:ic + 1])
                nc.vector.reciprocal(rsr[:, ic:ic + 1], rs[:, ic:ic + 1])
                # normalize Ex rows in place (bf16, cheap on DVE)
                nc.vector.tensor_scalar(E_sb[:, ic, :], E_sb[:, ic, :],
                                        rsr[:, ic:ic + 1], None, OP.mult)
            return E_sb

        def outp(n, E_sb):
            """out = Ex_norm^T @ H2"""
            H2s = h2s[n]
            for jc in range(EC):
                pO = ps_big.tile([P, D], F32, tag="big")
                for ic in range(EC):
                    nc.tensor.matmul(pO[:], E_sb[:, ic, jc * P:(jc + 1) * P],
                                     H2s[:, ic, :],
                                     start=(ic == 0), stop=(ic == EC - 1))
                o_sb = small.tile([P, D], BF16, tag="osb")
                nc.scalar.copy(o_sb[:], pO[:])
                nc.sync.dma_start(out[n, jc * P:(jc + 1) * P, :], o_sb[:])

        def body_all(_iv=None):
            # Interleave prep stages (weight-DMA bound) with H2 blocks
            # (prep-independent PE work) so the in-order PE queue never
            # stalls on a weight load.
            gens = [prep_type_stages(i) for i in range(NT)]
            next(gens[0])                   # W1[0] queued first
            _dma_split(nc, Wt2_sb[:], Wt2.rearrange("(c p) d -> p c d", p=P), 2)
            h2_one(0)
            next(gens[0]); h2_one(1)
            next(gens[0], None); h2_one(2)
            next(gens[1]); h2_one(3)
            next(gens[1]); h2_one(4)
            next(gens[1], None); h2_one(5)
            next(gens[2]); h2_one(6)
            next(gens[2]); h2_one(7)
            next(gens[2], None)
            # depth-2 software pipeline: LR/scores of graph n+2 are emitted
            # ahead of out(n) so the in-order PE queue always has independent
            # matmuls while the elementwise score chain of a graph drains.
            es = {0: scores(0, phase1(0)), 1: scores(1, phase1(1))}
            for n in range(NG):
                if n + 2 < NG:
                    es[n + 2] = scores(n + 2, phase1(n + 2))
                outp(n, es.pop(n))

        if reps == 1:
            body_all()
        else:
            with tc.For_i(0, reps, 1) as _iv:
                body_all(_iv)
    return nc


_NC_CACHE = {}
TRACE = False
_LAST = {}


def _get_nc():
    if "nc" not in _NC_CACHE:
        nc = bacc.Bacc("TRN2", target_bir_lowering=False, debug=False)
        build(nc)
        nc.compile()
        _NC_CACHE["nc"] = nc
    return _NC_CACHE["nc"]


def kernel(input_state, adj, entity_mask, query_vec, W_type, a_type,
           qattn_W1, qattn_W2):
    from concourse import bass_utils
    nc = _get_nc()
    bf = ml_dtypes.bfloat16
    input_state = np.asarray(input_state, dtype=np.float32)
    adj = np.asarray(adj, dtype=np.int32)
    query_vec = np.asarray(query_vec, dtype=np.float32)

    xT_all = np.ascontiguousarray(
        input_state.transpose(0, 2, 1)).astype(bf)              # [N, K, E]
    masks_all = np.ascontiguousarray(np.stack(
        [(adj == 0), (adj == 2) | (adj == 3), (adj == 3)], axis=1)).astype(np.uint8)
    qT_all = np.ascontiguousarray(query_vec.T).astype(bf)       # [K, N]
    at_h = np.ascontiguousarray(
        np.asarray(a_type, np.float32).reshape(NT, DC2, P).transpose(2, 1, 0))
    WtT_h = np.ascontiguousarray(
        np.asarray(W_type, np.float32).transpose(0, 2, 1)).astype(bf)
    Wt2_h = np.ascontiguousarray(np.asarray(W_type, np.float32)[2]).astype(bf)
    W1_h = np.ascontiguousarray(np.asarray(qattn_W1, np.float32)).astype(bf)
    W2q_h = np.ascontiguousarray(np.asarray(qattn_W2, np.float32)).astype(bf)

    in_maps = []
    for c in range(N_CORES):
        sl = slice(c * NG, (c + 1) * NG)
        in_maps.append({
            "xT": xT_all[sl], "masks": masks_all[sl],
            "qT": np.ascontiguousarray(qT_all[:, sl]),
            "at": at_h, "WtT": WtT_h, "Wt2": Wt2_h,
            "W1": W1_h, "W2q": W2q_h,
        })
    res = bass_utils.run_bass_kernel_spmd(nc, in_maps, core_ids=list(range(N_CORES)),
                                          trace=TRACE, stitch_traces=TRACE)
    _LAST["exec_ns"] = res.exec_time_ns
    _LAST["mean_ns"] = res.mean_exec_time_ns
    _LAST["trace"] = res.instructions_and_trace
    _LAST["scope_times"] = res.per_core_scope_times
    out = np.concatenate([np.asarray(r["out"]) for r in res.results], axis=0)
    return out.astype(np.float32)
